# revision 1
# baseline (speedup 1.0000x reference)
"""Trainium2 Bass kernel for nn_MultiHeadAttention (B=1, S=4096, d_model=768, 12 heads).

Sharding (8 cores): 2 head-groups (6 heads / 384 channels each) x 4 query-blocks
(1024 rows each).  Each core computes its head-group's contribution to the output
projection for its query block; the host sums the two head-group partials and adds
the constant bias terms (bk drops out of softmax; bv@Wo+bo added on host).

The shipped body is _emit_body_v2 (bf16 end-to-end), see its header comment:
pv reoriented to out[q,65] (full PE width + ones-column softmax denominator),
softmax exp split across ACT (exact exp) / DVE+Pool (two-term Schraudolph
bitcast exp whose summation is fused into the pv accumulation), normalization
folded into the psum evacuation, PE-transposed attn for the output projection,
and one-open-accumulation-group-per-PSUM-bank discipline throughout (HW
corrupts interleaved open groups sharing a bank).

The original f32r body (_emit_body, ~452 us) is kept as body="v1".
"""

import sys

sys.path.insert(0, "/opt/trn_rl_repo")

import numpy as np

import concourse.bass as bass
import concourse.mybir as mybir
from concourse.bass import ts, ds
from concourse.bass_utils import run_bass_kernel_spmd
from concourse.tile import TileContext

D_MODEL = 768
S = 4096
NH = 12
HD = 64
HG = 2                  # head groups
QB = 4                  # query blocks
C = D_MODEL // HG       # 384 channels per group
NHL = NH // HG          # 6 heads per group
QR = S // QB            # 1024 query rows per block
NCORES = 8
SCALE = float(1.0 / np.sqrt(np.float32(D_MODEL)))

F32 = mybir.dt.float32
F32R = mybir.dt.float32r
BF16 = mybir.dt.bfloat16
AF = mybir.ActivationFunctionType


def _r(ap):
    """View an fp32 AP as float32r for single-pass PE matmuls."""
    return ap.bitcast(F32R)


def _split_excess_waits(nc, max_waits=1):
    """walrus rejects instructions carrying more than one semaphore wait
    (setupSyncWait 'Too many sync wait commands').  Hoist excess waits onto
    no-op instructions inserted immediately before, on the same engine."""
    n_split = 0
    for f in nc.m.functions:
        for blk in f.blocks:
            new_insts = []
            for inst in blk.instructions:
                si = inst.sync_info
                if si is not None and si.on_wait and len(si.on_wait) > max_waits:
                    waits = list(si.on_wait)
                    keep = waits[-max_waits:]
                    extra = waits[:-max_waits]
                    for i in range(0, len(extra), max_waits):
                        chunk = extra[i : i + max_waits]
                        nop = mybir.InstNoOp(
                            name=f"{inst.name}_wsplit_{i}",
                            ins=[],
                            outs=[],
                            engine=inst.engine,
                            sync_info=mybir.SyncInfo(on_wait=chunk, on_update=[]),
                        )
                        new_insts.append(nop)
                        n_split += 1
                    si.on_wait = keep
                new_insts.append(inst)
            blk.instructions = new_insts
    return n_split


def _emit_body(nc, tc, io, use_f32r=True, stages=("proj", "attn", "oproj"), att_bf16=False,
               in_bf16=False, prof=None):
    QT, KT, VT, WQ, WK, WV, WO, BQ, OUT = (
        io["QT"], io["KT"], io["VT"], io["WQ"], io["WK"], io["WV"], io["WO"],
        io["BQ"], io["OUT"],
    )
    # float32r end-to-end: DRAM inputs are declared f32r, engine-produced
    # matmul operands are written as f32r (DVE/ACT round on write), so the
    # BIR verifier's "rounded to FP32r" rule is satisfied everywhere.
    DT = F32R if use_f32r else F32
    # attention-side dtype: bf16 halves nothing in cycle count but avoids the
    # slow f32r self-loading weight path and enables FWL on the PE
    DA = BF16 if att_bf16 else DT
    # input/projection-side dtype: bf16 halves the dominant input DMA traffic
    DI = BF16 if in_bf16 else DT

    consts = tc.alloc_tile_pool(name="consts", bufs=1)
    big = tc.alloc_tile_pool(name="big", bufs=1)

    # ---- weights -> SBUF ----
    wq_t, wk_t, wv_t = [], [], []
    for i in range(6):
        wq = consts.tile([128, C], DI, tag=f"wq{i}", name=f"wq{i}")
        d0 = nc.sync.dma_start(out=wq, in_=WQ[ts(i, 128), :])
        if prof is not None and i == 0:
            prof.snap(0, d0)
        wq_t.append(wq)
        wk = consts.tile([128, C], DI, tag=f"wk{i}", name=f"wk{i}")
        nc.sync.dma_start(out=wk, in_=WK[ts(i, 128), :])
        wk_t.append(wk)
        wv = consts.tile([128, C], DI, tag=f"wv{i}", name=f"wv{i}")
        nc.sync.dma_start(out=wv, in_=WV[ts(i, 128), :])
        wv_t.append(wv)
    wo_t = []
    for p in range(3):
        wo = consts.tile([128, D_MODEL], DT, tag=f"wo{p}", name=f"wo{p}")
        nc.sync.dma_start(out=wo, in_=WO[ts(p, 128), :])
        wo_t.append(wo)
    bq_t = []
    for p in range(3):
        bq = consts.tile([128, 1], F32, tag=f"bq{p}", name=f"bq{p}")
        nc.sync.dma_start(out=bq, in_=BQ[ts(p, 128)].rearrange("(p one) -> p one", one=1))
        bq_t.append(bq)
    ones64 = consts.tile([1, 64], F32, tag="ones64", name="ones64")
    nc.vector.memset(ones64, 1.0)
    # f32 ones source for the v_ext ones column (memset can't write f32r;
    # a DVE copy rounds f32 -> f32r)
    ones_col = consts.tile([128, NHL], F32, tag="ones_col", name="ones_col")
    nc.vector.memset(ones_col, 1.0)

    # ---- persistent activations ----
    qT_t = [big.tile([128, QR], DA, tag=f"qT{p}", name=f"qT{p}") for p in range(3)]
    kT_t = [big.tile([128, S], DA, tag=f"kT{p}", name=f"kT{p}") for p in range(3)]
    vext_t = [
        big.tile([128, NHL, HD + 1], DA, tag=f"vx{j}", name=f"vx{j}")
        for j in range(S // 128)
    ]

    # attention pools are allocated before the projection pools so the first
    # attention pair can interleave with vproj (LIFO release order).
    expp = tc.alloc_tile_pool(name="expp", bufs=2)
    attnp = tc.alloc_tile_pool(name="attnp", bufs=1)
    outp = tc.alloc_tile_pool(name="outp", bufs=1)
    smallp = tc.alloc_tile_pool(name="smallp", bufs=1)
    psS = tc.alloc_tile_pool(name="psS", bufs=2, space="PSUM")
    psV = tc.alloc_tile_pool(name="psV", bufs=2, space="PSUM")

    do_attn = "attn" in stages
    do_oproj = "oproj" in stages
    NKT = S // 128          # 32 key tiles
    GK = 2                  # key tiles per exp group

    def attn_pair_begin(qc, p):
        at = attnp.tile([128, 512], DT, tag=f"attn{qc}_{p}", name=f"attn{qc}_{p}")
        pvh = [
            psV.tile([HD + 1, 512], F32, tag="pv", name=f"pv{qc}_{p}_{h}")
            for h in range(2)
        ]
        return at, pvh

    def attn_group(qc, p, pvh, grp, offload_h1=True):
        es = []
        for h in range(2):
            sp = psS.tile([128, GK, 512], F32, tag="psS", name=f"sp{qc}_{p}_{grp}_{h}")
            for kt in range(GK):
                j = grp * GK + kt
                nc.tensor.matmul(
                    sp[:, kt, :],
                    lhsT=kT_t[p][ds(64 * h, 64), ts(j, 128)],
                    rhs=qT_t[p][ds(64 * h, 64), ts(qc, 512)],
                    start=True, stop=True,
                )
            e = expp.tile([128, GK, 512], DA, tag="exp", name=f"e{qc}_{p}_{grp}_{h}")
            if h == 0 or not offload_h1:
                # ACT reads PSUM at ~2.3 cyc/elem (vs 1.17 from SBUF); split
                # the softmax between ACT-direct and a DVE evacuation +
                # ACT-from-SBUF to balance the engines.
                nc.scalar.activation(e, sp, AF.Exp)
            else:
                s_sb = expp.tile([128, GK, 512], F32, tag="s_sb",
                                 name=f"ssb{qc}_{p}_{grp}_{h}", bufs=2)
                nc.vector.tensor_copy(s_sb, sp)
                nc.scalar.activation(e, s_sb, AF.Exp)
            es.append(e)
        for h in range(2):
            for kt in range(GK):
                j = grp * GK + kt
                nc.tensor.matmul(
                    pvh[h],
                    lhsT=vext_t[j][:, p * 2 + h, :],
                    rhs=es[h][:, kt, :],
                    start=(j == 0), stop=(j == NKT - 1),
                )

    def attn_pair_end(qc, p, at, pvh):
        for h in range(2):
            rr = smallp.tile([1, 512], F32, tag="rr", name=f"rr{qc}_{p}_{h}")
            nc.vector.reciprocal(rr, pvh[h][ds(HD, 1), :])
            rbc = psV.tile([64, 512], F32, tag="pv", name=f"rbc{qc}_{p}_{h}")
            nc.tensor.matmul(rbc, lhsT=ones64, rhs=rr, start=True, stop=True)
            pv_sb = smallp.tile([64, 512], F32, tag="pv_sb", name=f"pvsb{qc}_{p}_{h}")
            nc.vector.tensor_copy(pv_sb, pvh[h][ds(0, HD), :])
            gate = nc.vector.tensor_mul(at[ds(64 * h, 64), :], pv_sb, rbc)
            if prof is not None and h == 1:
                prof.snap(4 + qc * 3 + p, gate)
        return at

    # ================= phase A: projections =================
    instream = tc.alloc_tile_pool(name="instream", bufs=2)
    psA = tc.alloc_tile_pool(name="psA", bufs=2, space="PSUM")

    # kproj: kT = WK^T KT
    for ck in range(S // 512):
        kin = [instream.tile([128, 512], DI, tag=f"xin{i}", name=f"kin{i}_{ck}", bufs=2) for i in range(6)]
        for i in range(6):
            nc.sync.dma_start(out=kin[i], in_=KT[ts(i, 128), ts(ck, 512)])
        for p in range(3):
            ps = psA.tile([128, 512], F32, tag="psA", name=f"ps_k{p}_{ck}")
            for i in range(6):
                nc.tensor.matmul(
                    ps, lhsT=wk_t[i][:, ts(p, 128)], rhs=kin[i],
                    start=(i == 0), stop=(i == 5),
                )
            gate = nc.vector.tensor_copy(kT_t[p][:, ts(ck, 512)], ps)
            if prof is not None and ck == S // 512 - 1 and p == 2:
                prof.snap(2, gate)

    # qproj: qT = (WQ^T QT) * s + bq*s
    for qc in range(QR // 512):
        qin = [instream.tile([128, 512], DI, tag=f"xin{i}", name=f"qin{i}_{qc}", bufs=2) for i in range(6)]
        for i in range(6):
            nc.sync.dma_start(out=qin[i], in_=QT[ts(i, 128), ts(qc, 512)])
        for p in range(3):
            ps = psA.tile([128, 512], F32, tag="psA", name=f"ps_q{p}_{qc}")
            for i in range(6):
                nc.tensor.matmul(
                    ps, lhsT=wq_t[i][:, ts(p, 128)], rhs=qin[i],
                    start=(i == 0), stop=(i == 5),
                )
            gate = nc.scalar.activation(
                qT_t[p][:, ts(qc, 512)], ps, AF.Identity, bias=bq_t[p], scale=SCALE
            )
            if prof is not None and qc == QR // 512 - 1 and p == 2:
                prof.snap(1, gate)

    # vproj: v[key, ch] = sum_in VT[in, key] WV[in, ch], written per-head with a
    # ones column appended (lhsT for the pv matmul).  The first attention pair
    # (qc0, p0) is interleaved here: its scores/exp need only qT/kT (already
    # done) and its pv consumes v_ext key tiles right as vproj produces them,
    # so ACT/DVE softmax work hides under the DMA-bound vproj window.
    pair00 = attn_pair_begin(0, 0) if do_attn else None
    for ck in range(S // 512):
        vin = [instream.tile([128, 512], DI, tag=f"xin{i}", name=f"vin{i}_{ck}", bufs=2) for i in range(6)]
        for i in range(6):
            nc.sync.dma_start(out=vin[i], in_=VT[ts(i, 128), ts(ck, 512)])
        for ksub in range(4):
            j = ck * 4 + ksub
            ps = psA.tile([128, C], F32, tag="psA", name=f"ps_v{j}")
            for i in range(6):
                nc.tensor.matmul(
                    ps, lhsT=vin[i][:, ts(ksub, 128)], rhs=wv_t[i],
                    start=(i == 0), stop=(i == 5),
                )
            nc.vector.tensor_copy(
                vext_t[j][:, :, 0:HD], ps.rearrange("p (h d) -> p h d", h=NHL)
            )
            gate = nc.vector.tensor_copy(vext_t[j][:, :, HD], ones_col)
            if prof is not None and j == S // 128 - 1:
                prof.snap(3, gate)
        if do_attn:
            for grp in (2 * ck, 2 * ck + 1):
                attn_group(0, 0, pair00[1], grp)

    psA.release()
    instream.release()

    # ================= phase B: attention =================
    for qc in range(QR // 512):
        attn_tiles = []
        for p in range(3):
            if not do_attn:
                break
            if qc == 0 and p == 0:
                # already computed interleaved with vproj; just normalize
                at, pvh = pair00
            else:
                at, pvh = attn_pair_begin(qc, p)
                for grp in range(NKT // GK):
                    attn_group(qc, p, pvh, grp)
            attn_tiles.append(at)
            attn_pair_end(qc, p, at, pvh)
        # oproj for this q chunk: OUT[qc*512 + qs*128 .. , :] partial
        for qs in range(4):
            if not (do_attn and do_oproj):
                break
            ob = outp.tile([128, D_MODEL], F32, tag="ob", name=f"ob{qc}_{qs}")
            for oc in range(2):
                po = psV.tile([128, 384], F32, tag="pv", name=f"po{qc}_{qs}_{oc}")
                for p in range(3):
                    nc.tensor.matmul(
                        po,
                        lhsT=attn_tiles[p][:, ts(qs, 128)],
                        rhs=wo_t[p][:, ts(oc, 384)],
                        start=(p == 0), stop=(p == 2),
                    )
                nc.vector.tensor_copy(ob[:, ts(oc, 384)], po)
            gate = nc.sync.dma_start(out=OUT[ds(qc * 512 + qs * 128, 128), :], in_=ob)
            if prof is not None and qs == 3:
                prof.snap(10 + qc, gate)

    for pool in [psV, psS, smallp, outp, attnp, expp, big, consts]:
        pool.release()


# ======================= v2 body =======================
#
# Differences from v1:
#  * bf16 end-to-end: inputs, weights, qT/kT/vext, exp tiles, attnT, Wo.
#  * pv matmul reoriented: out[q 128, 65] (64 ch + ones col = softmax denom),
#    full-width M=128, N=65 -> half the PE cycles of the [65, 512] version.
#  * softmax exp spread over ACT / DVE / Pool via per-tile "paths":
#      A: ACT exp PSUM -> SBUF bf16
#      B: evac (Pool/DVE) PSUM -> bf16, ACT exp from SBUF
#      C: evac + DVE two-term Schraudolph (2x tensor_scalar + tensor_add)
#      D: DVE single-term Schraudolph direct from PSUM
#      F: evac + DVE single-term Schraudolph
#      G: evac + Pool single-term Schraudolph
#    Schraudolph exp: bitcast(round(s*A + B)) as bf16 ~= 2^(s*log2 e) with a
#    mantissa-periodic ripple (+-3.3% single / +-1.2% two-term).  Constant
#    factors cancel in softmax (denominator sums the same approximations);
#    B offsets are ripple-centered so A/B tiles and C/D/F/G tiles carry the
#    same mean weight.
#  * softmax denominator via ones column of vext; normalization folded into
#    the pv evacuation (per-partition scale), so no rbc/broadcast matmuls.
#  * attn transposed back to [ch, q] with PE transpose (f32r identity) for
#    the output projection lhsT.
SCH_A = float(128.0 / np.log(2.0))
SCH_B1 = 16149.22          # two-term leg 1 (mean-centered pair)
SCH_B2 = 16086.82          # two-term leg 2 (~1/2 ripple period offset)
SCH_B0 = 16248.67          # single-term, mean-centered
I16 = mybir.dt.int16

V2_DEFAULT_CFG = dict(
    # 32 chars, one per (grp, h) tile of a pair; tuned from ubench rates:
    #   ACT ~0.92 ns/row any source; DVE psum 0.98, bf16-ts 0.37, bf16-tt 0.55.
    # GPSIMD (Pool) cannot touch PSUM, so every tile's PSUM egress is via ACT
    # (path A: direct exp) or DVE (evac); Pool only post-processes SBUF tiles.
    #   A = ACT exp direct; Q = DVE evac + Schraudolph ts legs (Pool/DVE),
    #       two-term add FUSED into pv (pv(t1)+pv(t2));
    #   M/N/H = two-term with explicit add (legacy mix).
    paths="AQAQAQAA" "AQAQAQAA" "AQAQAQAA" "AQAQAQAA",
    evac="dve",         # PSUM->SBUF evacuation engine for non-A paths
    norm="act",         # pv normalize+evac engine ("act" | "dve")
    attnT_evac="dve",   # psT -> attnT copy
    kevac="act",        # kproj evacuation
    vevac="dve",        # vproj evacuation
    oevac="dve",        # oproj evacuation
    fp8_proj=False,     # fp8e4 DoubleRow projections: 243us in CoreSim but
                        # rel_err 4.5e-2 on HW (over the 2e-2 gate) -- needs
                        # e4m3-variant/layout debugging before enabling
    interleave00=True,
)


def _v2_engine(nc, name):
    return {"pool": nc.gpsimd, "dve": nc.vector, "act": nc.scalar}[name]


def _v2_copy(nc, eng, out, in_):
    if eng is nc.scalar:
        return eng.copy(out, in_)
    return eng.tensor_copy(out, in_)


def _emit_body_v2(nc, tc, io, cfg, prof=None):
    QT, KT, VT, WQ, WK, WV, WO, BQ, EYE, OUT = (
        io["QT"], io["KT"], io["VT"], io["WQ"], io["WK"], io["WV"], io["WO"],
        io["BQ"], io["EYE"], io["OUT"],
    )
    BF = BF16
    ev = _v2_engine(nc, cfg["evac"])
    kev = _v2_engine(nc, cfg["kevac"])
    vev = _v2_engine(nc, cfg["vevac"])
    oev = _v2_engine(nc, cfg["oevac"])
    aev = _v2_engine(nc, cfg["attnT_evac"])

    consts = tc.alloc_tile_pool(name="consts", bufs=1)
    big = tc.alloc_tile_pool(name="big", bufs=1)

    fp8 = bool(cfg.get("fp8_proj"))
    FP8 = mybir.dt.float8e4
    WDT = FP8 if fp8 else BF
    NW = 3 if fp8 else 6          # weight/input k-tiles (256-deep if fp8)
    def _wsrc(W, i):
        if fp8:
            return W[ds(256 * i, 256), :].rearrange("(two p) c -> p two c", two=2)
        return W[ts(i, 128), :]
    wshape = [128, 2, C] if fp8 else [128, C]
    wq_t, wk_t, wv_t = [], [], []
    for i in range(NW):
        wk = consts.tile(wshape, WDT, tag=f"wk{i}", name=f"wk{i}")
        d0 = nc.sync.dma_start(out=wk, in_=_wsrc(WK, i))
        if prof is not None and i == 0:
            prof.snap(0, d0)
        wk_t.append(wk)
    for i in range(NW):
        wq = consts.tile(wshape, WDT, tag=f"wq{i}", name=f"wq{i}")
        nc.scalar.dma_start(out=wq, in_=_wsrc(WQ, i))
        wq_t.append(wq)
    # the remaining constants are consumed late (vproj/qproj/pair-end);
    # allocate now, DMA after the first kproj chunk so the input stream
    # isn't stuck behind them in the queues
    wv_t = [consts.tile(wshape, WDT, tag=f"wv{i}", name=f"wv{i}")
            for i in range(NW)]
    wo_t = [consts.tile([128, D_MODEL], BF, tag=f"wo{p}", name=f"wo{p}")
            for p in range(3)]
    bq_t = [consts.tile([128, 1], F32, tag=f"bq{p}", name=f"bq{p}")
            for p in range(3)]
    eye = consts.tile([128, 128], F32R, tag="eye", name="eye")

    def _emit_late_const_dmas():
        for i in range(NW):
            nc.scalar.dma_start(out=wv_t[i], in_=_wsrc(WV, i))
        for p in range(3):
            nc.sync.dma_start(out=wo_t[p], in_=WO[ts(p, 128), :])
            nc.scalar.dma_start(
                out=bq_t[p],
                in_=BQ[ts(p, 128)].rearrange("(p one) -> p one", one=1))
        nc.sync.dma_start(out=eye, in_=EYE[:, :])
    ones6 = consts.tile([128, NHL], BF, tag="ones6", name="ones6")
    nc.vector.memset(ones6, 1.0)

    qT_t = [big.tile([128, QR], BF, tag=f"qT{p}", name=f"qT{p}") for p in range(3)]
    kT_t = [big.tile([128, S], BF, tag=f"kT{p}", name=f"kT{p}") for p in range(3)]
    vext_t = [
        big.tile([128, NHL, HD + 1], BF, tag=f"vx{j}", name=f"vx{j}")
        for j in range(S // 128)
    ]

    # attention pools before projection pools (LIFO release order)
    expp = tc.alloc_tile_pool(name="expp", bufs=2)
    attnp = tc.alloc_tile_pool(name="attnp", bufs=2)
    smallp = tc.alloc_tile_pool(name="smallp", bufs=2)
    attnTp = tc.alloc_tile_pool(name="attnTp", bufs=3)
    outp = tc.alloc_tile_pool(name="outp", bufs=2)
    psS = tc.alloc_tile_pool(name="psS", bufs=2, space="PSUM")
    psPV = tc.alloc_tile_pool(name="psPV", bufs=2, space="PSUM")

    NKT = S // 128
    GK = 8

    def len_legs(tile_idx):
        return 2 if cfg["paths"][tile_idx % 32] == "Q" else 1

    def emit_exp_legs(tile_idx, nm, sp):
        """Consume a scores psum tile; return the list of bf16 lhsT tiles
        whose pv contributions must be summed (1 for exact exp, 2 for the
        fused two-term Schraudolph, where pv(t1)+pv(t2) == pv(t1+t2))."""
        path = cfg["paths"][tile_idx % 32]
        if path == "A":
            e = expp.tile([128, GK, 128], BF, tag="e", name=f"e{nm}", bufs=4)
            nc.scalar.activation(e, sp, AF.Exp)
            return [e]
        if path == "Q":
            ssb = expp.tile([128, GK, 128], BF, tag="ssb", name=f"ssb{nm}", bufs=2)
            nc.vector.tensor_copy(ssb, sp)
            t1 = expp.tile([128, GK, 128], BF, tag="t1", name=f"t1{nm}", bufs=3)
            t2 = expp.tile([128, GK, 128], BF, tag="t2", name=f"t2{nm}", bufs=3)
            if cfg.get("q_legs_dve"):
                leg1 = leg2 = nc.vector
            else:
                leg1 = nc.gpsimd
                leg2 = nc.vector if tile_idx % 4 == 3 else nc.gpsimd
            leg1.tensor_scalar(
                t1.bitcast(I16), ssb, SCH_A, SCH_B1, mybir.AluOpType.mult,
                mybir.AluOpType.add)
            leg2.tensor_scalar(
                t2.bitcast(I16), ssb, SCH_A, SCH_B2, mybir.AluOpType.mult,
                mybir.AluOpType.add)
            return [t1, t2]
        return [emit_exp(tile_idx, nm, sp)]

    def emit_exp(tile_idx, nm, sp):
        path = cfg["paths"][tile_idx % 32]
        e = expp.tile([128, GK, 128], BF, tag="e", name=f"e{nm}", bufs=4)
        if path == "A":
            nc.scalar.activation(e, sp, AF.Exp)
            return e
        if path == "D":
            nc.vector.tensor_scalar(
                e.bitcast(I16), sp, SCH_A, SCH_B0, mybir.AluOpType.mult,
                mybir.AluOpType.add)
            return e
        ssb = expp.tile([128, GK, 128], BF, tag="ssb", name=f"ssb{nm}", bufs=2)
        _v2_copy(nc, ev, ssb, sp)
        if path == "B":
            nc.scalar.activation(e, ssb, AF.Exp)
        elif path in ("C", "H", "M", "N"):
            # two-term Schraudolph: legs ~1/2 period apart, summed.
            # Pool tt is slow (1.7 ns/row), so the add always goes to DVE
            # except in the (unused) pure-Pool C path.
            leg1 = nc.vector if path == "H" else nc.gpsimd
            leg2 = nc.gpsimd if path in ("C", "M") else nc.vector
            fin = nc.gpsimd if path == "C" else nc.vector
            t1 = expp.tile([128, GK, 128], BF, tag="t1", name=f"t1{nm}", bufs=2)
            t2 = expp.tile([128, GK, 128], BF, tag="t2", name=f"t2{nm}", bufs=2)
            leg1.tensor_scalar(
                t1.bitcast(I16), ssb, SCH_A, SCH_B1, mybir.AluOpType.mult,
                mybir.AluOpType.add)
            leg2.tensor_scalar(
                t2.bitcast(I16), ssb, SCH_A, SCH_B2, mybir.AluOpType.mult,
                mybir.AluOpType.add)
            fin.tensor_add(e, t1, t2)
        elif path == "F":
            nc.vector.tensor_scalar(
                e.bitcast(I16), ssb, SCH_A, SCH_B0, mybir.AluOpType.mult,
                mybir.AluOpType.add)
        elif path == "G":
            nc.gpsimd.tensor_scalar(
                e.bitcast(I16), ssb, SCH_A, SCH_B0, mybir.AluOpType.mult,
                mybir.AluOpType.add)
        else:
            raise ValueError(path)
        return e

    # PSUM rule learned on HW: a bank may hold at most ONE open accumulation
    # group; other matmuls writing the same bank while a group is open corrupt
    # it.  So attention runs in 128-query subchunks (u): per (u, h) the pv
    # accumulation owns its bank exclusively until stop, scores tiles are
    # closed single-matmul writes in the psS banks.
    GKS = 8                  # key tiles per scores/exp tile at N=128

    def attn_pair(qc, p):
        attnT = attnTp.tile([128, QB, 128], BF, tag="attnT", name=f"aT{qc}_{p}")
        gate = None
        NG = NKT // GKS
        for u in range(4):
            pvh = [
                psPV.tile([128, 512], F32, tag=f"pv{h}", name=f"pv{qc}_{p}_{u}_{h}",
                          bufs=1)
                for h in range(2)
            ]
            # total pv matmuls per h (for start/stop flags); legs counted
            nlegs = [sum(len_legs((u * 4 + g) * 2 + h) for g in range(NG)) * GKS
                     for h in range(2)]
            ndone = [0, 0]
            pending = []          # (grp, h, leg_tiles)

            def flush_pv(upto):
                while pending and pending[0][0] < upto:
                    g, h, legs = pending.pop(0)
                    for leg in legs:
                        for kt in range(GKS):
                            j = g * GKS + kt
                            nc.tensor.matmul(
                                pvh[h][:, 0:HD + 1],
                                lhsT=leg[:, kt, :],
                                rhs=vext_t[j][:, p * 2 + h, :],
                                start=(ndone[h] == 0),
                                stop=(ndone[h] == nlegs[h] - 1),
                            )
                            ndone[h] += 1

            for grp in range(NG):
                for h in range(2):
                    sp = psS.tile([128, GKS, 128], F32, tag="psS",
                                  name=f"sp{qc}_{p}_{u}_{grp}_{h}")
                    for kt in range(GKS):
                        j = grp * GKS + kt
                        nc.tensor.matmul(
                            sp[:, kt, :],
                            lhsT=kT_t[p][ds(64 * h, 64), ts(j, 128)],
                            rhs=qT_t[p][ds(64 * h, 64), ds(qc * 512 + u * 128, 128)],
                            start=True, stop=True,
                        )
                    legs = emit_exp_legs((u * 4 + grp) * 2 + h,
                                         f"{qc}_{p}_{u}_{grp}_{h}", sp)
                    pending.append((grp, h, legs))
                # pv runs two key-groups behind scores so the exp chain's
                # latency hides under the next groups' PE work
                flush_pv(grp - 1)
            flush_pv(NG)
            for h in range(2):
                rr = smallp.tile([128, 1], F32, tag="rr", name=f"rr{qc}_{p}_{u}_{h}")
                nc.vector.reciprocal(rr, pvh[h][:, ds(HD, 1)])
                asb = attnp.tile([128, HD], F32R, tag="asb",
                                 name=f"asb{qc}_{p}_{u}_{h}", bufs=2)
                if cfg["norm"] == "act":
                    nc.scalar.activation(asb, pvh[h][:, 0:HD], AF.Copy, scale=rr)
                else:
                    nc.vector.tensor_scalar(
                        asb, pvh[h][:, 0:HD], rr, None, mybir.AluOpType.mult)
                # transpose target lives in the unused upper half of the pv
                # bank: its accumulation group is closed by now, and all
                # writes to this bank come from the in-order PE.
                psT = pvh[h].bitcast(F32R)[0:64, ds(384, 128)]
                nc.tensor.transpose(psT, asb, eye)
                gate = _v2_copy(nc, aev, attnT[ds(64 * h, 64), u, :], psT)
        if prof is not None:
            prof.snap(4 + qc * 3 + p, gate)
        return attnT

    # ================= phase A: projections =================
    instream = tc.alloc_tile_pool(name="instream", bufs=2)
    psA = tc.alloc_tile_pool(name="psA", bufs=2, space="PSUM")

    xshape = [128, 2, 512] if fp8 else [128, 512]
    PM = mybir.MatmulPerfMode.DoubleRow if fp8 else None
    def _xsrc(X, i, ck):
        if fp8:
            return X[ds(256 * i, 256), ts(ck, 512)].rearrange(
                "(two p) c -> p two c", two=2)
        return X[ts(i, 128), ts(ck, 512)]
    def _wslice(w, p):
        return w[:, :, ts(p, 128)] if fp8 else w[:, ts(p, 128)]
    for ck in range(S // 512):
        kin = [instream.tile(xshape, WDT, tag=f"xin{i}", name=f"kin{i}_{ck}", bufs=2) for i in range(NW)]
        for i in range(NW):
            dq = nc.sync if i % 2 == 0 else nc.scalar
            dq.dma_start(out=kin[i], in_=_xsrc(KT, i, ck))
        for p in range(3):
            ps = psA.tile([128, 512], F32, tag="psA", name=f"ps_k{p}_{ck}")
            for i in range(NW):
                nc.tensor.matmul(
                    ps, lhsT=_wslice(wk_t[i], p), rhs=kin[i],
                    start=(i == 0), stop=(i == NW - 1), perf_mode=PM,
                )
            gate = _v2_copy(nc, kev, kT_t[p][:, ts(ck, 512)], ps)
            if prof is not None and ck == S // 512 - 1 and p == 2:
                prof.snap(2, gate)
        if ck == 0:
            _emit_late_const_dmas()

    for qc in range(QR // 512):
        qin = [instream.tile(xshape, WDT, tag=f"xin{i}", name=f"qin{i}_{qc}", bufs=2) for i in range(NW)]
        for i in range(NW):
            dq = nc.sync if i % 2 == 0 else nc.scalar
            dq.dma_start(out=qin[i], in_=_xsrc(QT, i, qc))
        for p in range(3):
            ps = psA.tile([128, 512], F32, tag="psA", name=f"ps_q{p}_{qc}")
            for i in range(NW):
                nc.tensor.matmul(
                    ps, lhsT=_wslice(wq_t[i], p), rhs=qin[i],
                    start=(i == 0), stop=(i == NW - 1), perf_mode=PM,
                )
            gate = nc.scalar.activation(
                qT_t[p][:, ts(qc, 512)], ps, AF.Identity, bias=bq_t[p], scale=SCALE
            )
            if prof is not None and qc == QR // 512 - 1 and p == 2:
                prof.snap(1, gate)

    for ck in range(S // 512):
        vin = [instream.tile(xshape, WDT, tag=f"xin{i}", name=f"vin{i}_{ck}", bufs=2) for i in range(NW)]
        for i in range(NW):
            dq = nc.sync if i % 2 == 0 else nc.scalar
            dq.dma_start(out=vin[i], in_=_xsrc(VT, i, ck))
        for ksub in range(4):
            j = ck * 4 + ksub
            ps = psA.tile([128, 512], F32, tag="psA", name=f"ps_v{j}")
            for i in range(NW):
                vsl = vin[i][:, :, ts(ksub, 128)] if fp8 else vin[i][:, ts(ksub, 128)]
                nc.tensor.matmul(
                    ps[:, 0:C], lhsT=vsl, rhs=wv_t[i],
                    start=(i == 0), stop=(i == NW - 1), perf_mode=PM,
                )
            _v2_copy(nc, vev,
                vext_t[j][:, :, 0:HD], ps[:, 0:C].rearrange("p (h d) -> p h d", h=NHL))
            gate = _v2_copy(nc, vev, vext_t[j][:, :, HD], ones6)
            if prof is not None and j == S // 128 - 1:
                prof.snap(3, gate)

    psA.release()
    instream.release()

    # psO reuses the PSUM banks psA freed (LIFO arena)
    psO = tc.alloc_tile_pool(name="psO", bufs=2, space="PSUM")

    # ================= phase B: attention =================
    for qc in range(QR // 512):
        attnTs = []
        for p in range(3):
            attnTs.append(attn_pair(qc, p))
        for qs in range(4):
            ob = outp.tile([128, D_MODEL], F32, tag="ob", name=f"ob{qc}_{qs}")
            for oc in range(2):
                po = psO.tile([128, 512], F32, tag="po", name=f"po{qc}_{qs}_{oc}")
                for p in range(3):
                    nc.tensor.matmul(
                        po[:, 0:384],
                        lhsT=attnTs[p][:, qs, :],
                        rhs=wo_t[p][:, ts(oc, 384)],
                        start=(p == 0), stop=(p == 2),
                    )
                _v2_copy(nc, oev, ob[:, ts(oc, 384)], po[:, 0:384])
            gate = nc.sync.dma_start(out=OUT[ds(qc * 512 + qs * 128, 128), :], in_=ob)
            if prof is not None and qs == 3:
                prof.snap(10 + qc, gate)

    for pool in [psO, psPV, psS, outp, attnTp, smallp, attnp, expp, big, consts]:
        pool.release()


_nc_cache = {}


PROF_LK = 256           # ladder length (ticks)
PROF_TICK_CYC = 4800    # NX cycles per tick  (~4us at 1.2 GHz)
PROF_NSNAP = 12


class _Prof:
    """On-device sampling profiler: a GPSIMD tick ladder (sequencer-only
    stores + fixed-cycle nops, invisible to Tile's dep tracker) plus snapshot
    DMAs of the tick buffer gated on phase-completion instructions."""

    def __init__(self, nc, prog_ap, PROG):
        self.nc = nc
        self.prog_ap = prog_ap
        self.PROG = PROG

    def snap(self, idx, gate):
        from concourse.tile_rust import add_dep_helper
        d = self.nc.sync.dma_start(out=self.PROG[ds(idx, 1), :], in_=self.prog_ap)
        add_dep_helper(d.ins, gate.ins, sync=True, reason=f"prof snap {idx}")


def _emit_prof_ladder(nc, prog_ap):
    """Emit (post-Tile) the Pool tick ladder, then relocate it to just after
    Pool's preamble-barrier instructions so it runs concurrently with the
    kernel body."""
    ladder = []
    reg_ctx = nc.gpsimd.register("prof_tick")
    reg = reg_ctx.__enter__()
    z = nc.gpsimd.reg_alu(reg, 0, 0, mybir.AluOpType.add)
    ladder.append(z.ins)
    for i in range(PROF_LK):
        s = nc.gpsimd.store(prog_ap[0:1, ds(i, 1)], reg)
        ladder.append(s.ins)
    for i in range(PROF_LK):
        a = nc.gpsimd.reg_alu(reg, reg, 1, mybir.AluOpType.add)
        ladder.append(a.ins)
        s = nc.gpsimd.store(prog_ap[0:1, ds(i, 1)], reg)
        ladder.append(s.ins)
        n = nc.gpsimd.nop(cycle_cnt=PROF_TICK_CYC, nofuse=True)
        ladder.append(n.ins)
    ladder_set = set(id(x) for x in ladder)
    f = nc.m.functions[0]
    # remove from wherever they were appended
    for blk in f.blocks:
        blk.instructions = [x for x in blk.instructions if id(x) not in ladder_set]
    # insert at the start of the TileContext body block so Pool runs the
    # ladder concurrently with the kernel (Pool is otherwise unused there)
    for blk in f.blocks:
        if blk.name.startswith("tile_context"):
            blk.instructions[0:0] = ladder
            return
    raise RuntimeError("profiler: no tile_context block found for tick ladder")


def build_nc(reps=1, use_f32r=True, split_waits=True, stages=("proj", "attn", "oproj"),
             timing_mode=False, att_bf16=False, in_bf16=False, profile_ladder=False,
             body="v2", cfg_over=None):
    cfg = dict(V2_DEFAULT_CFG)
    if profile_ladder and body == "v2":
        # the tick ladder owns the Pool engine: route all Pool work to DVE.
        # The measured time is therefore an upper bound of the production
        # kernel (which spreads the same work over one more engine).
        cfg["paths"] = (cfg["paths"].replace("G", "F").replace("C", "H")
                        .replace("M", "H").replace("N", "H"))
        cfg["q_legs_dve"] = True
    if cfg_over:
        cfg.update(cfg_over)
    key = (reps, use_f32r, split_waits, tuple(stages), timing_mode, att_bf16, in_bf16,
           profile_ladder, body, tuple(sorted(cfg.items())))
    if key in _nc_cache:
        return _nc_cache[key]
    nc = bass.Bass()
    if body == "v2":
        DI = mybir.dt.float8e4 if cfg.get("fp8_proj") else BF16
        DA = BF16
        DW = BF16
    else:
        DT = F32R if use_f32r else F32
        DA = BF16 if att_bf16 else DT
        DI = BF16 if in_bf16 else DT
        DW = DT
    io = {
        "QT": nc.declare_dram_parameter("QT", [D_MODEL, QR], DI, isOutput=False),
        "KT": nc.declare_dram_parameter("KT", [D_MODEL, S], DI, isOutput=False),
        "VT": nc.declare_dram_parameter("VT", [D_MODEL, S], DI, isOutput=False),
        "WQ": nc.declare_dram_parameter("WQ", [D_MODEL, C], DI, isOutput=False),
        "WK": nc.declare_dram_parameter("WK", [D_MODEL, C], DI, isOutput=False),
        "WV": nc.declare_dram_parameter("WV", [D_MODEL, C], DI, isOutput=False),
        "WO": nc.declare_dram_parameter("WO", [C, D_MODEL], DW, isOutput=False),
        "BQ": nc.declare_dram_parameter("BQ", [C], F32, isOutput=False),
        "OUT": nc.declare_dram_parameter("OUT", [QR, D_MODEL], F32, isOutput=True),
    }
    if body == "v2":
        io["EYE"] = nc.declare_dram_parameter("EYE", [128, 128], F32R, isOutput=False)
        if cfg.get("debug"):
            io["DBG"] = nc.declare_dram_parameter("DBG", [128, 1024], F32,
                                                  isOutput=True)
    prof = None
    prog_ap = None
    if profile_ladder:
        PROG = nc.declare_dram_parameter(
            "PROG", [PROF_NSNAP, PROF_LK], mybir.dt.int32, isOutput=True)
        prog_ap = nc.alloc_sbuf_tensor("prog_ticks", [1, PROF_LK], mybir.dt.int32).ap()
        prof = _Prof(nc, prog_ap, PROG)
    with TileContext(nc) as tc:
        for _ in range(reps):
            if body == "v2":
                _emit_body_v2(nc, tc, io, cfg, prof=prof)
            else:
                _emit_body(nc, tc, io, use_f32r=use_f32r, stages=stages,
                           att_bf16=att_bf16, in_bf16=in_bf16, prof=prof)
    if profile_ladder:
        _emit_prof_ladder(nc, prog_ap)
    if split_waits:
        _split_excess_waits(nc)
    _nc_cache[key] = nc
    return nc


def fp8_proj_enabled():
    return bool(V2_DEFAULT_CFG.get("fp8_proj"))


def make_in_maps(Q, K, V, Wq, bq, Wk, bk, Wv, bv, Wo, bo, in_bf16=False,
                 body="v2"):
    """Host-side sharding.  Returns (in_maps, host_const) where host_const is
    the [768] vector added to every output row (bv @ Wo + bo)."""
    Qm = np.asarray(Q, np.float32).reshape(S, D_MODEL)
    Km = np.asarray(K, np.float32).reshape(S, D_MODEL)
    Vm = np.asarray(V, np.float32).reshape(S, D_MODEL)
    QT = np.ascontiguousarray(Qm.T)
    KT = np.ascontiguousarray(Km.T)
    VT = np.ascontiguousarray(Vm.T)
    Wq = np.asarray(Wq, np.float32); Wk = np.asarray(Wk, np.float32)
    Wv = np.asarray(Wv, np.float32); Wo = np.asarray(Wo, np.float32)
    bq = np.asarray(bq, np.float32); bv = np.asarray(bv, np.float32)
    bo = np.asarray(bo, np.float32)

    import ml_dtypes
    if body == "v2":
        in_bf16 = True
    fp8_proj = body == "v2" and fp8_proj_enabled()
    def cvt(a):
        if fp8_proj:
            return np.ascontiguousarray(a).astype(ml_dtypes.float8_e4m3)
        return np.ascontiguousarray(a).astype(ml_dtypes.bfloat16) if in_bf16 \
            else np.ascontiguousarray(a)
    def cvt_w(a):
        if body == "v2":
            return np.ascontiguousarray(a).astype(ml_dtypes.bfloat16)
        return np.ascontiguousarray(a)
    in_maps = []
    for c in range(NCORES):
        g, b = divmod(c, QB)
        ch = slice(g * C, (g + 1) * C)
        m = {
            "QT": cvt(QT[:, b * QR : (b + 1) * QR]),
            "KT": cvt(KT),
            "VT": cvt(VT),
            "WQ": cvt(Wq[:, ch]),
            "WK": cvt(Wk[:, ch]),
            "WV": cvt(Wv[:, ch]),
            "WO": cvt_w(Wo[ch, :]),
            # device computes qT = psum*SCALE + BQ, so prescale the bias here
            "BQ": np.ascontiguousarray(bq[ch] * np.float32(SCALE)),
        }
        if body == "v2":
            m["EYE"] = np.eye(128, dtype=np.float32)
        in_maps.append(m)
    host_const = (bv @ Wo + bo).astype(np.float32)
    return in_maps, host_const


def kernel(Q, K, V, Wq, bq, Wk, bk, Wv, bv, Wo, bo):
    nc = build_nc()
    in_maps, host_const = make_in_maps(Q, K, V, Wq, bq, Wk, bk, Wv, bv, Wo, bo)
    res = run_bass_kernel_spmd(nc, in_maps, core_ids=list(range(NCORES)))
    out = np.zeros((S, D_MODEL), np.float32)
    for c in range(NCORES):
        g, b = divmod(c, QB)
        out[b * QR : (b + 1) * QR, :] += res.results[c]["OUT"]
    out += host_const[None, :]
    return out.reshape(1, S, D_MODEL)



# revision 21
# speedup vs baseline: 1.0109x; 1.0109x over previous
"""Trainium2 Bass kernel for nn_MultiHeadAttention (B=1, S=4096, d_model=768, 12 heads).

Sharding (8 cores): 2 head-groups (6 heads / 384 channels each) x 4 query-blocks
(1024 rows each).  Each core computes its head-group's contribution to the output
projection for its query block; the host sums the two head-group partials and adds
the constant bias terms (bk drops out of softmax; bv@Wo+bo added on host).

The shipped body is _emit_body_v2 (bf16 end-to-end), see its header comment:
pv reoriented to out[q,65] (full PE width + ones-column softmax denominator),
softmax exp split across ACT (exact exp) / DVE+Pool (two-term Schraudolph
bitcast exp whose summation is fused into the pv accumulation), normalization
folded into the psum evacuation, PE-transposed attn for the output projection,
and one-open-accumulation-group-per-PSUM-bank discipline throughout (HW
corrupts interleaved open groups sharing a bank).

The original f32r body (_emit_body, ~452 us) is kept as body="v1".
"""

import sys

sys.path.insert(0, "/opt/trn_rl_repo")

import numpy as np

import concourse.bass as bass
import concourse.mybir as mybir
from concourse.bass import ts, ds
from concourse.bass_utils import run_bass_kernel_spmd
from concourse.tile import TileContext

D_MODEL = 768
S = 4096
NH = 12
HD = 64
HG = 2                  # head groups
QB = 4                  # query blocks
C = D_MODEL // HG       # 384 channels per group
NHL = NH // HG          # 6 heads per group
QR = S // QB            # 1024 query rows per block
NCORES = 8
SCALE = float(1.0 / np.sqrt(np.float32(D_MODEL)))

F32 = mybir.dt.float32
F32R = mybir.dt.float32r
BF16 = mybir.dt.bfloat16
AF = mybir.ActivationFunctionType


def _r(ap):
    """View an fp32 AP as float32r for single-pass PE matmuls."""
    return ap.bitcast(F32R)


def _split_excess_waits(nc, max_waits=1):
    """walrus rejects instructions carrying more than one semaphore wait
    (setupSyncWait 'Too many sync wait commands').  Hoist excess waits onto
    no-op instructions inserted immediately before, on the same engine."""
    n_split = 0
    for f in nc.m.functions:
        for blk in f.blocks:
            new_insts = []
            for inst in blk.instructions:
                si = inst.sync_info
                if si is not None and si.on_wait and len(si.on_wait) > max_waits:
                    waits = list(si.on_wait)
                    keep = waits[-max_waits:]
                    extra = waits[:-max_waits]
                    for i in range(0, len(extra), max_waits):
                        chunk = extra[i : i + max_waits]
                        nop = mybir.InstNoOp(
                            name=f"{inst.name}_wsplit_{i}",
                            ins=[],
                            outs=[],
                            engine=inst.engine,
                            sync_info=mybir.SyncInfo(on_wait=chunk, on_update=[]),
                        )
                        new_insts.append(nop)
                        n_split += 1
                    si.on_wait = keep
                new_insts.append(inst)
            blk.instructions = new_insts
    return n_split


def _emit_body(nc, tc, io, use_f32r=True, stages=("proj", "attn", "oproj"), att_bf16=False,
               in_bf16=False, prof=None):
    QT, KT, VT, WQ, WK, WV, WO, BQ, OUT = (
        io["QT"], io["KT"], io["VT"], io["WQ"], io["WK"], io["WV"], io["WO"],
        io["BQ"], io["OUT"],
    )
    # float32r end-to-end: DRAM inputs are declared f32r, engine-produced
    # matmul operands are written as f32r (DVE/ACT round on write), so the
    # BIR verifier's "rounded to FP32r" rule is satisfied everywhere.
    DT = F32R if use_f32r else F32
    # attention-side dtype: bf16 halves nothing in cycle count but avoids the
    # slow f32r self-loading weight path and enables FWL on the PE
    DA = BF16 if att_bf16 else DT
    # input/projection-side dtype: bf16 halves the dominant input DMA traffic
    DI = BF16 if in_bf16 else DT

    consts = tc.alloc_tile_pool(name="consts", bufs=1)
    big = tc.alloc_tile_pool(name="big", bufs=1)

    # ---- weights -> SBUF ----
    wq_t, wk_t, wv_t = [], [], []
    for i in range(6):
        wq = consts.tile([128, C], DI, tag=f"wq{i}", name=f"wq{i}")
        d0 = nc.sync.dma_start(out=wq, in_=WQ[ts(i, 128), :])
        if prof is not None and i == 0:
            prof.snap(0, d0)
        wq_t.append(wq)
        wk = consts.tile([128, C], DI, tag=f"wk{i}", name=f"wk{i}")
        nc.sync.dma_start(out=wk, in_=WK[ts(i, 128), :])
        wk_t.append(wk)
        wv = consts.tile([128, C], DI, tag=f"wv{i}", name=f"wv{i}")
        nc.sync.dma_start(out=wv, in_=WV[ts(i, 128), :])
        wv_t.append(wv)
    wo_t = []
    for p in range(3):
        wo = consts.tile([128, D_MODEL], DT, tag=f"wo{p}", name=f"wo{p}")
        nc.sync.dma_start(out=wo, in_=WO[ts(p, 128), :])
        wo_t.append(wo)
    bq_t = []
    for p in range(3):
        bq = consts.tile([128, 1], F32, tag=f"bq{p}", name=f"bq{p}")
        nc.sync.dma_start(out=bq, in_=BQ[ts(p, 128)].rearrange("(p one) -> p one", one=1))
        bq_t.append(bq)
    ones64 = consts.tile([1, 64], F32, tag="ones64", name="ones64")
    nc.vector.memset(ones64, 1.0)
    # f32 ones source for the v_ext ones column (memset can't write f32r;
    # a DVE copy rounds f32 -> f32r)
    ones_col = consts.tile([128, NHL], F32, tag="ones_col", name="ones_col")
    nc.vector.memset(ones_col, 1.0)

    # ---- persistent activations ----
    qT_t = [big.tile([128, QR], DA, tag=f"qT{p}", name=f"qT{p}") for p in range(3)]
    kT_t = [big.tile([128, S], DA, tag=f"kT{p}", name=f"kT{p}") for p in range(3)]
    vext_t = [
        big.tile([128, NHL, HD + 1], DA, tag=f"vx{j}", name=f"vx{j}")
        for j in range(S // 128)
    ]

    # attention pools are allocated before the projection pools so the first
    # attention pair can interleave with vproj (LIFO release order).
    expp = tc.alloc_tile_pool(name="expp", bufs=2)
    attnp = tc.alloc_tile_pool(name="attnp", bufs=1)
    outp = tc.alloc_tile_pool(name="outp", bufs=1)
    smallp = tc.alloc_tile_pool(name="smallp", bufs=1)
    psS = tc.alloc_tile_pool(name="psS", bufs=2, space="PSUM")
    psV = tc.alloc_tile_pool(name="psV", bufs=2, space="PSUM")

    do_attn = "attn" in stages
    do_oproj = "oproj" in stages
    NKT = S // 128          # 32 key tiles
    GK = 2                  # key tiles per exp group

    def attn_pair_begin(qc, p):
        at = attnp.tile([128, 512], DT, tag=f"attn{qc}_{p}", name=f"attn{qc}_{p}")
        pvh = [
            psV.tile([HD + 1, 512], F32, tag="pv", name=f"pv{qc}_{p}_{h}")
            for h in range(2)
        ]
        return at, pvh

    def attn_group(qc, p, pvh, grp, offload_h1=True):
        es = []
        for h in range(2):
            sp = psS.tile([128, GK, 512], F32, tag="psS", name=f"sp{qc}_{p}_{grp}_{h}")
            for kt in range(GK):
                j = grp * GK + kt
                nc.tensor.matmul(
                    sp[:, kt, :],
                    lhsT=kT_t[p][ds(64 * h, 64), ts(j, 128)],
                    rhs=qT_t[p][ds(64 * h, 64), ts(qc, 512)],
                    start=True, stop=True,
                )
            e = expp.tile([128, GK, 512], DA, tag="exp", name=f"e{qc}_{p}_{grp}_{h}")
            if h == 0 or not offload_h1:
                # ACT reads PSUM at ~2.3 cyc/elem (vs 1.17 from SBUF); split
                # the softmax between ACT-direct and a DVE evacuation +
                # ACT-from-SBUF to balance the engines.
                nc.scalar.activation(e, sp, AF.Exp)
            else:
                s_sb = expp.tile([128, GK, 512], F32, tag="s_sb",
                                 name=f"ssb{qc}_{p}_{grp}_{h}", bufs=2)
                nc.vector.tensor_copy(s_sb, sp)
                nc.scalar.activation(e, s_sb, AF.Exp)
            es.append(e)
        for h in range(2):
            for kt in range(GK):
                j = grp * GK + kt
                nc.tensor.matmul(
                    pvh[h],
                    lhsT=vext_t[j][:, p * 2 + h, :],
                    rhs=es[h][:, kt, :],
                    start=(j == 0), stop=(j == NKT - 1),
                )

    def attn_pair_end(qc, p, at, pvh):
        for h in range(2):
            rr = smallp.tile([1, 512], F32, tag="rr", name=f"rr{qc}_{p}_{h}")
            nc.vector.reciprocal(rr, pvh[h][ds(HD, 1), :])
            rbc = psV.tile([64, 512], F32, tag="pv", name=f"rbc{qc}_{p}_{h}")
            nc.tensor.matmul(rbc, lhsT=ones64, rhs=rr, start=True, stop=True)
            pv_sb = smallp.tile([64, 512], F32, tag="pv_sb", name=f"pvsb{qc}_{p}_{h}")
            nc.vector.tensor_copy(pv_sb, pvh[h][ds(0, HD), :])
            gate = nc.vector.tensor_mul(at[ds(64 * h, 64), :], pv_sb, rbc)
            if prof is not None and h == 1:
                prof.snap(4 + qc * 3 + p, gate)
        return at

    # ================= phase A: projections =================
    instream = tc.alloc_tile_pool(name="instream", bufs=2)
    psA = tc.alloc_tile_pool(name="psA", bufs=2, space="PSUM")

    # kproj: kT = WK^T KT
    for ck in range(S // 512):
        kin = [instream.tile([128, 512], DI, tag=f"xin{i}", name=f"kin{i}_{ck}", bufs=2) for i in range(6)]
        for i in range(6):
            nc.sync.dma_start(out=kin[i], in_=KT[ts(i, 128), ts(ck, 512)])
        for p in range(3):
            ps = psA.tile([128, 512], F32, tag="psA", name=f"ps_k{p}_{ck}")
            for i in range(6):
                nc.tensor.matmul(
                    ps, lhsT=wk_t[i][:, ts(p, 128)], rhs=kin[i],
                    start=(i == 0), stop=(i == 5),
                )
            gate = nc.vector.tensor_copy(kT_t[p][:, ts(ck, 512)], ps)
            if prof is not None and ck == S // 512 - 1 and p == 2:
                prof.snap(2, gate)

    # qproj: qT = (WQ^T QT) * s + bq*s
    for qc in range(QR // 512):
        qin = [instream.tile([128, 512], DI, tag=f"xin{i}", name=f"qin{i}_{qc}", bufs=2) for i in range(6)]
        for i in range(6):
            nc.sync.dma_start(out=qin[i], in_=QT[ts(i, 128), ts(qc, 512)])
        for p in range(3):
            ps = psA.tile([128, 512], F32, tag="psA", name=f"ps_q{p}_{qc}")
            for i in range(6):
                nc.tensor.matmul(
                    ps, lhsT=wq_t[i][:, ts(p, 128)], rhs=qin[i],
                    start=(i == 0), stop=(i == 5),
                )
            gate = nc.scalar.activation(
                qT_t[p][:, ts(qc, 512)], ps, AF.Identity, bias=bq_t[p], scale=SCALE
            )
            if prof is not None and qc == QR // 512 - 1 and p == 2:
                prof.snap(1, gate)

    # vproj: v[key, ch] = sum_in VT[in, key] WV[in, ch], written per-head with a
    # ones column appended (lhsT for the pv matmul).  The first attention pair
    # (qc0, p0) is interleaved here: its scores/exp need only qT/kT (already
    # done) and its pv consumes v_ext key tiles right as vproj produces them,
    # so ACT/DVE softmax work hides under the DMA-bound vproj window.
    pair00 = attn_pair_begin(0, 0) if do_attn else None
    for ck in range(S // 512):
        vin = [instream.tile([128, 512], DI, tag=f"xin{i}", name=f"vin{i}_{ck}", bufs=2) for i in range(6)]
        for i in range(6):
            nc.sync.dma_start(out=vin[i], in_=VT[ts(i, 128), ts(ck, 512)])
        for ksub in range(4):
            j = ck * 4 + ksub
            ps = psA.tile([128, C], F32, tag="psA", name=f"ps_v{j}")
            for i in range(6):
                nc.tensor.matmul(
                    ps, lhsT=vin[i][:, ts(ksub, 128)], rhs=wv_t[i],
                    start=(i == 0), stop=(i == 5),
                )
            nc.vector.tensor_copy(
                vext_t[j][:, :, 0:HD], ps.rearrange("p (h d) -> p h d", h=NHL)
            )
            gate = nc.vector.tensor_copy(vext_t[j][:, :, HD], ones_col)
            if prof is not None and j == S // 128 - 1:
                prof.snap(3, gate)
        if do_attn:
            for grp in (2 * ck, 2 * ck + 1):
                attn_group(0, 0, pair00[1], grp)

    psA.release()
    instream.release()

    # ================= phase B: attention =================
    for qc in range(QR // 512):
        attn_tiles = []
        for p in range(3):
            if not do_attn:
                break
            if qc == 0 and p == 0:
                # already computed interleaved with vproj; just normalize
                at, pvh = pair00
            else:
                at, pvh = attn_pair_begin(qc, p)
                for grp in range(NKT // GK):
                    attn_group(qc, p, pvh, grp)
            attn_tiles.append(at)
            attn_pair_end(qc, p, at, pvh)
        # oproj for this q chunk: OUT[qc*512 + qs*128 .. , :] partial
        for qs in range(4):
            if not (do_attn and do_oproj):
                break
            ob = outp.tile([128, D_MODEL], F32, tag="ob", name=f"ob{qc}_{qs}")
            for oc in range(2):
                po = psV.tile([128, 384], F32, tag="pv", name=f"po{qc}_{qs}_{oc}")
                for p in range(3):
                    nc.tensor.matmul(
                        po,
                        lhsT=attn_tiles[p][:, ts(qs, 128)],
                        rhs=wo_t[p][:, ts(oc, 384)],
                        start=(p == 0), stop=(p == 2),
                    )
                nc.vector.tensor_copy(ob[:, ts(oc, 384)], po)
            gate = nc.sync.dma_start(out=OUT[ds(qc * 512 + qs * 128, 128), :], in_=ob)
            if prof is not None and qs == 3:
                prof.snap(10 + qc, gate)

    for pool in [psV, psS, smallp, outp, attnp, expp, big, consts]:
        pool.release()


# ======================= v2 body =======================
#
# Differences from v1:
#  * bf16 end-to-end: inputs, weights, qT/kT/vext, exp tiles, attnT, Wo.
#  * pv matmul reoriented: out[q 128, 65] (64 ch + ones col = softmax denom),
#    full-width M=128, N=65 -> half the PE cycles of the [65, 512] version.
#  * softmax exp spread over ACT / DVE / Pool via per-tile "paths":
#      A: ACT exp PSUM -> SBUF bf16
#      B: evac (Pool/DVE) PSUM -> bf16, ACT exp from SBUF
#      C: evac + DVE two-term Schraudolph (2x tensor_scalar + tensor_add)
#      D: DVE single-term Schraudolph direct from PSUM
#      F: evac + DVE single-term Schraudolph
#      G: evac + Pool single-term Schraudolph
#    Schraudolph exp: bitcast(round(s*A + B)) as bf16 ~= 2^(s*log2 e) with a
#    mantissa-periodic ripple (+-3.3% single / +-1.2% two-term).  Constant
#    factors cancel in softmax (denominator sums the same approximations);
#    B offsets are ripple-centered so A/B tiles and C/D/F/G tiles carry the
#    same mean weight.
#  * softmax denominator via ones column of vext; normalization folded into
#    the pv evacuation (per-partition scale), so no rbc/broadcast matmuls.
#  * attn transposed back to [ch, q] with PE transpose (f32r identity) for
#    the output projection lhsT.
SCH_A = float(128.0 / np.log(2.0))
SCH_B1 = 16149.22          # two-term leg 1 (mean-centered pair)
SCH_B2 = 16086.82          # two-term leg 2 (~1/2 ripple period offset)
SCH_B0 = 16248.67          # single-term, mean-centered
I16 = mybir.dt.int16

V2_DEFAULT_CFG = dict(
    # 32 chars, one per (grp, h) tile of a pair; tuned from ubench rates:
    #   ACT ~0.92 ns/row any source; DVE psum 0.98, bf16-ts 0.37, bf16-tt 0.55.
    # GPSIMD (Pool) cannot touch PSUM, so every tile's PSUM egress is via ACT
    # (path A: direct exp) or DVE (evac); Pool only post-processes SBUF tiles.
    #   A = ACT exp direct; Q = DVE evac + Schraudolph ts legs (Pool/DVE),
    #       two-term add FUSED into pv (pv(t1)+pv(t2));
    #   M/N/H = two-term with explicit add (legacy mix).
    paths="AQAQAQAA" "AQAQAQAA" "AQAQAQAA" "AQAQAQAA",
    evac="dve",         # PSUM->SBUF evacuation engine for non-A paths
    norm="act",         # pv normalize+evac engine ("act" | "dve")
    attnT_evac="dve",   # psT -> attnT copy
    kevac="act",        # kproj evacuation
    vevac="dve",        # vproj evacuation
    oevac="dve",        # oproj evacuation
    fp8_proj=False,     # fp8e4 DoubleRow projections: 243us in CoreSim but
                        # rel_err 4.5e-2 on HW (over the 2e-2 gate) -- needs
                        # e4m3-variant/layout debugging before enabling
    interleave00=True,
)


def _v2_engine(nc, name):
    return {"pool": nc.gpsimd, "dve": nc.vector, "act": nc.scalar}[name]


def _v2_copy(nc, eng, out, in_):
    if eng is nc.scalar:
        return eng.copy(out, in_)
    return eng.tensor_copy(out, in_)


def _emit_body_v2(nc, tc, io, cfg, prof=None):
    QT, KT, VT, WQ, WK, WV, WO, BQ, EYE, OUT = (
        io["QT"], io["KT"], io["VT"], io["WQ"], io["WK"], io["WV"], io["WO"],
        io["BQ"], io["EYE"], io["OUT"],
    )
    BF = BF16
    ev = _v2_engine(nc, cfg["evac"])
    kev = _v2_engine(nc, cfg["kevac"])
    vev = _v2_engine(nc, cfg["vevac"])
    oev = _v2_engine(nc, cfg["oevac"])
    aev = _v2_engine(nc, cfg["attnT_evac"])

    consts = tc.alloc_tile_pool(name="consts", bufs=1)
    big = tc.alloc_tile_pool(name="big", bufs=1)

    fp8 = bool(cfg.get("fp8_proj"))
    FP8 = mybir.dt.float8e4
    WDT = FP8 if fp8 else BF
    NW = 3 if fp8 else 6          # weight/input k-tiles (256-deep if fp8)
    def _wsrc(W, i):
        if fp8:
            return W[ds(256 * i, 256), :].rearrange("(two p) c -> p two c", two=2)
        return W[ts(i, 128), :]
    wshape = [128, 2, C] if fp8 else [128, C]
    wq_t, wk_t, wv_t = [], [], []
    for i in range(NW):
        wk = consts.tile(wshape, WDT, tag=f"wk{i}", name=f"wk{i}")
        d0 = nc.sync.dma_start(out=wk, in_=_wsrc(WK, i))
        if prof is not None and i == 0:
            prof.snap(0, d0)
        wk_t.append(wk)
    for i in range(NW):
        wq = consts.tile(wshape, WDT, tag=f"wq{i}", name=f"wq{i}")
        nc.scalar.dma_start(out=wq, in_=_wsrc(WQ, i))
        wq_t.append(wq)
    # the remaining constants are consumed late (vproj/qproj/pair-end);
    # allocate now, DMA after the first kproj chunk so the input stream
    # isn't stuck behind them in the queues
    wv_t = [consts.tile(wshape, WDT, tag=f"wv{i}", name=f"wv{i}")
            for i in range(NW)]
    wo_t = [consts.tile([128, D_MODEL], BF, tag=f"wo{p}", name=f"wo{p}")
            for p in range(3)]
    bq_t = [consts.tile([128, 1], F32, tag=f"bq{p}", name=f"bq{p}")
            for p in range(3)]
    eye = consts.tile([128, 128], F32R, tag="eye", name="eye")

    def _emit_late_const_dmas():
        for i in range(NW):
            nc.scalar.dma_start(out=wv_t[i], in_=_wsrc(WV, i))
        for p in range(3):
            nc.sync.dma_start(out=wo_t[p], in_=WO[ts(p, 128), :])
            nc.scalar.dma_start(
                out=bq_t[p],
                in_=BQ[ts(p, 128)].rearrange("(p one) -> p one", one=1))
        nc.sync.dma_start(out=eye, in_=EYE[:, :])
    ones6 = consts.tile([128, NHL], BF, tag="ones6", name="ones6")
    nc.vector.memset(ones6, 1.0)

    qT_t = [big.tile([128, QR], BF, tag=f"qT{p}", name=f"qT{p}") for p in range(3)]
    kT_t = [big.tile([128, S], BF, tag=f"kT{p}", name=f"kT{p}") for p in range(3)]
    vext_t = [
        big.tile([128, NHL, HD + 1], BF, tag=f"vx{j}", name=f"vx{j}")
        for j in range(S // 128)
    ]

    # attention pools before projection pools (LIFO release order)
    expp = tc.alloc_tile_pool(name="expp", bufs=2)
    attnp = tc.alloc_tile_pool(name="attnp", bufs=2)
    smallp = tc.alloc_tile_pool(name="smallp", bufs=2)
    attnTp = tc.alloc_tile_pool(name="attnTp", bufs=3)
    outp = tc.alloc_tile_pool(name="outp", bufs=2)
    psS = tc.alloc_tile_pool(name="psS", bufs=2, space="PSUM")
    psPV = tc.alloc_tile_pool(name="psPV", bufs=2, space="PSUM")

    NKT = S // 128
    GK = 8

    def len_legs(tile_idx):
        return 2 if cfg["paths"][tile_idx % 32] == "Q" else 1

    def emit_exp_legs(tile_idx, nm, sp):
        """Consume a scores psum tile; return the list of bf16 lhsT tiles
        whose pv contributions must be summed (1 for exact exp, 2 for the
        fused two-term Schraudolph, where pv(t1)+pv(t2) == pv(t1+t2))."""
        path = cfg["paths"][tile_idx % 32]
        if path == "A":
            e = expp.tile([128, GK, 128], BF, tag="e", name=f"e{nm}", bufs=4)
            nc.scalar.activation(e, sp, AF.Exp)
            return [e]
        if path == "Q":
            ssb = expp.tile([128, GK, 128], BF, tag="ssb", name=f"ssb{nm}", bufs=2)
            nc.vector.tensor_copy(ssb, sp)
            t1 = expp.tile([128, GK, 128], BF, tag="t1", name=f"t1{nm}", bufs=3)
            t2 = expp.tile([128, GK, 128], BF, tag="t2", name=f"t2{nm}", bufs=3)
            if cfg.get("q_legs_dve"):
                leg1 = leg2 = nc.vector
            else:
                leg1 = nc.gpsimd
                leg2 = nc.vector if tile_idx % 4 == 3 else nc.gpsimd
            leg1.tensor_scalar(
                t1.bitcast(I16), ssb, SCH_A, SCH_B1, mybir.AluOpType.mult,
                mybir.AluOpType.add)
            leg2.tensor_scalar(
                t2.bitcast(I16), ssb, SCH_A, SCH_B2, mybir.AluOpType.mult,
                mybir.AluOpType.add)
            return [t1, t2]
        return [emit_exp(tile_idx, nm, sp)]

    def emit_exp(tile_idx, nm, sp):
        path = cfg["paths"][tile_idx % 32]
        e = expp.tile([128, GK, 128], BF, tag="e", name=f"e{nm}", bufs=4)
        if path == "A":
            nc.scalar.activation(e, sp, AF.Exp)
            return e
        if path == "D":
            nc.vector.tensor_scalar(
                e.bitcast(I16), sp, SCH_A, SCH_B0, mybir.AluOpType.mult,
                mybir.AluOpType.add)
            return e
        ssb = expp.tile([128, GK, 128], BF, tag="ssb", name=f"ssb{nm}", bufs=2)
        _v2_copy(nc, ev, ssb, sp)
        if path == "B":
            nc.scalar.activation(e, ssb, AF.Exp)
        elif path in ("C", "H", "M", "N"):
            # two-term Schraudolph: legs ~1/2 period apart, summed.
            # Pool tt is slow (1.7 ns/row), so the add always goes to DVE
            # except in the (unused) pure-Pool C path.
            leg1 = nc.vector if path == "H" else nc.gpsimd
            leg2 = nc.gpsimd if path in ("C", "M") else nc.vector
            fin = nc.gpsimd if path == "C" else nc.vector
            t1 = expp.tile([128, GK, 128], BF, tag="t1", name=f"t1{nm}", bufs=2)
            t2 = expp.tile([128, GK, 128], BF, tag="t2", name=f"t2{nm}", bufs=2)
            leg1.tensor_scalar(
                t1.bitcast(I16), ssb, SCH_A, SCH_B1, mybir.AluOpType.mult,
                mybir.AluOpType.add)
            leg2.tensor_scalar(
                t2.bitcast(I16), ssb, SCH_A, SCH_B2, mybir.AluOpType.mult,
                mybir.AluOpType.add)
            fin.tensor_add(e, t1, t2)
        elif path == "F":
            nc.vector.tensor_scalar(
                e.bitcast(I16), ssb, SCH_A, SCH_B0, mybir.AluOpType.mult,
                mybir.AluOpType.add)
        elif path == "G":
            nc.gpsimd.tensor_scalar(
                e.bitcast(I16), ssb, SCH_A, SCH_B0, mybir.AluOpType.mult,
                mybir.AluOpType.add)
        else:
            raise ValueError(path)
        return e

    # PSUM rule learned on HW: a bank may hold at most ONE open accumulation
    # group; other matmuls writing the same bank while a group is open corrupt
    # it.  So attention runs in 128-query subchunks (u): per (u, h) the pv
    # accumulation owns its bank exclusively until stop, scores tiles are
    # closed single-matmul writes in the psS banks.
    GKS = 8                  # key tiles per scores/exp tile at N=128

    def attn_pair(qc, p):
        attnT = attnTp.tile([128, QB, 128], BF, tag="attnT", name=f"aT{qc}_{p}")
        gate = None
        NG = NKT // GKS
        for u in range(4):
            pvh = [
                psPV.tile([128, 512], F32, tag=f"pv{h}", name=f"pv{qc}_{p}_{u}_{h}",
                          bufs=1)
                for h in range(2)
            ]
            # total pv matmuls per h (for start/stop flags); legs counted
            nlegs = [sum(len_legs((u * 4 + g) * 2 + h) for g in range(NG)) * GKS
                     for h in range(2)]
            ndone = [0, 0]
            pending = []          # (grp, h, leg_tiles)

            def flush_pv(upto):
                while pending and pending[0][0] < upto:
                    g, h, legs = pending.pop(0)
                    for leg in legs:
                        for kt in range(GKS):
                            j = g * GKS + kt
                            nc.tensor.matmul(
                                pvh[h][:, 0:HD + 1],
                                lhsT=leg[:, kt, :],
                                rhs=vext_t[j][:, p * 2 + h, :],
                                start=(ndone[h] == 0),
                                stop=(ndone[h] == nlegs[h] - 1),
                            )
                            ndone[h] += 1

            for grp in range(NG):
                for h in range(2):
                    sp = psS.tile([128, GKS, 128], F32, tag="psS",
                                  name=f"sp{qc}_{p}_{u}_{grp}_{h}")
                    for kt in range(GKS):
                        j = grp * GKS + kt
                        nc.tensor.matmul(
                            sp[:, kt, :],
                            lhsT=kT_t[p][ds(64 * h, 64), ts(j, 128)],
                            rhs=qT_t[p][ds(64 * h, 64), ds(qc * 512 + u * 128, 128)],
                            start=True, stop=True,
                        )
                    legs = emit_exp_legs((u * 4 + grp) * 2 + h,
                                         f"{qc}_{p}_{u}_{grp}_{h}", sp)
                    pending.append((grp, h, legs))
                # pv runs two key-groups behind scores so the exp chain's
                # latency hides under the next groups' PE work
                flush_pv(grp - 1)
            flush_pv(NG)
            for h in range(2):
                rr = smallp.tile([128, 1], F32, tag="rr", name=f"rr{qc}_{p}_{u}_{h}")
                nc.vector.reciprocal(rr, pvh[h][:, ds(HD, 1)])
                asb = attnp.tile([128, HD], F32R, tag="asb",
                                 name=f"asb{qc}_{p}_{u}_{h}", bufs=2)
                if cfg["norm"] == "act":
                    nc.scalar.activation(asb, pvh[h][:, 0:HD], AF.Copy, scale=rr)
                else:
                    nc.vector.tensor_scalar(
                        asb, pvh[h][:, 0:HD], rr, None, mybir.AluOpType.mult)
                # transpose target lives in the unused upper half of the pv
                # bank: its accumulation group is closed by now, and all
                # writes to this bank come from the in-order PE.
                psT = pvh[h].bitcast(F32R)[0:64, ds(384, 128)]
                nc.tensor.transpose(psT, asb, eye)
                gate = _v2_copy(nc, aev, attnT[ds(64 * h, 64), u, :], psT)
        if prof is not None:
            prof.snap(4 + qc * 3 + p, gate)
        return attnT

    # ================= phase A: projections =================
    instream = tc.alloc_tile_pool(name="instream", bufs=2)
    psA = tc.alloc_tile_pool(name="psA", bufs=2, space="PSUM")

    xshape = [128, 2, 512] if fp8 else [128, 512]
    PM = mybir.MatmulPerfMode.DoubleRow if fp8 else None
    def _xsrc(X, i, ck):
        if fp8:
            return X[ds(256 * i, 256), ts(ck, 512)].rearrange(
                "(two p) c -> p two c", two=2)
        return X[ts(i, 128), ts(ck, 512)]
    def _wslice(w, p):
        return w[:, :, ts(p, 128)] if fp8 else w[:, ts(p, 128)]
    for ck in range(S // 512):
        kin = [instream.tile(xshape, WDT, tag=f"xin{i}", name=f"kin{i}_{ck}", bufs=2) for i in range(NW)]
        for i in range(NW):
            dq = nc.sync if i % 2 == 0 else nc.scalar
            dq.dma_start(out=kin[i], in_=_xsrc(KT, i, ck))
        for p in range(3):
            ps = psA.tile([128, 512], F32, tag="psA", name=f"ps_k{p}_{ck}")
            for i in range(NW):
                nc.tensor.matmul(
                    ps, lhsT=_wslice(wk_t[i], p), rhs=kin[i],
                    start=(i == 0), stop=(i == NW - 1), perf_mode=PM,
                )
            gate = _v2_copy(nc, kev, kT_t[p][:, ts(ck, 512)], ps)
            if prof is not None and ck == S // 512 - 1 and p == 2:
                prof.snap(2, gate)
        if ck == 0:
            _emit_late_const_dmas()

    for qc in range(QR // 512):
        qin = [instream.tile(xshape, WDT, tag=f"xin{i}", name=f"qin{i}_{qc}", bufs=2) for i in range(NW)]
        for i in range(NW):
            dq = nc.sync if i % 2 == 0 else nc.scalar
            dq.dma_start(out=qin[i], in_=_xsrc(QT, i, qc))
        for p in range(3):
            ps = psA.tile([128, 512], F32, tag="psA", name=f"ps_q{p}_{qc}")
            for i in range(NW):
                nc.tensor.matmul(
                    ps, lhsT=_wslice(wq_t[i], p), rhs=qin[i],
                    start=(i == 0), stop=(i == NW - 1), perf_mode=PM,
                )
            gate = nc.scalar.activation(
                qT_t[p][:, ts(qc, 512)], ps, AF.Identity, bias=bq_t[p], scale=SCALE
            )
            if prof is not None and qc == QR // 512 - 1 and p == 2:
                prof.snap(1, gate)

    for ck in range(S // 512):
        vin = [instream.tile(xshape, WDT, tag=f"xin{i}", name=f"vin{i}_{ck}", bufs=2) for i in range(NW)]
        for i in range(NW):
            dq = nc.sync if i % 2 == 0 else nc.scalar
            dq.dma_start(out=vin[i], in_=_xsrc(VT, i, ck))
        for ksub in range(4):
            j = ck * 4 + ksub
            ps = psA.tile([128, 512], F32, tag="psA", name=f"ps_v{j}")
            for i in range(NW):
                vsl = vin[i][:, :, ts(ksub, 128)] if fp8 else vin[i][:, ts(ksub, 128)]
                nc.tensor.matmul(
                    ps[:, 0:C], lhsT=vsl, rhs=wv_t[i],
                    start=(i == 0), stop=(i == NW - 1), perf_mode=PM,
                )
            _v2_copy(nc, vev,
                vext_t[j][:, :, 0:HD], ps[:, 0:C].rearrange("p (h d) -> p h d", h=NHL))
            gate = _v2_copy(nc, vev, vext_t[j][:, :, HD], ones6)
            if prof is not None and j == S // 128 - 1:
                prof.snap(3, gate)

    psA.release()
    instream.release()

    # psO reuses the PSUM banks psA freed (LIFO arena)
    psO = tc.alloc_tile_pool(name="psO", bufs=2, space="PSUM")

    # ================= phase B: attention =================
    for qc in range(QR // 512):
        attnTs = []
        for p in range(3):
            attnTs.append(attn_pair(qc, p))
        for qs in range(4):
            ob = outp.tile([128, D_MODEL], F32, tag="ob", name=f"ob{qc}_{qs}")
            for oc in range(2):
                po = psO.tile([128, 512], F32, tag="po", name=f"po{qc}_{qs}_{oc}")
                for p in range(3):
                    nc.tensor.matmul(
                        po[:, 0:384],
                        lhsT=attnTs[p][:, qs, :],
                        rhs=wo_t[p][:, ts(oc, 384)],
                        start=(p == 0), stop=(p == 2),
                    )
                _v2_copy(nc, oev, ob[:, ts(oc, 384)], po[:, 0:384])
            gate = nc.sync.dma_start(out=OUT[ds(qc * 512 + qs * 128, 128), :], in_=ob)
            if prof is not None and qs == 3:
                prof.snap(10 + qc, gate)

    for pool in [psO, psPV, psS, outp, attnTp, smallp, attnp, expp, big, consts]:
        pool.release()


# ======================= v3 body =======================
#
# Differences from v2 (motivated by the HW/CoreSim gap: CoreSim charges no
# LDWEIGHTS and ~2ns/instr decode, yet HW pairs ran ~44-52us vs ~33us sim —
# consistent with ~1500 scores MMs (lhsT swapped EVERY matmul) and ~2100 pv
# MMs (lhsT = exp tile, also swapped every matmul) paying unhidden weight
# loads and dispatch):
#  * scores: kT[h, j] stationary, qT streamed at N=512 -> 64 MMs/pair
#    (vs 256), each LDW hidden under the other head-half's 213ns stream
#    (row groups 0-63 / 64-127 alternate).
#  * pv reoriented to out[65, 512q]: vext[j] is the STATIONARY operand and
#    the exp tile streams as rhs at N=512 -> 64 MMs/pair (vs ~352).  The
#    ones column of vext still yields the softmax denominator in row 64.
#    PSUM: one [128,512] bank per h, accumulation group open across all 32
#    key tiles (sole group in its bank).
#  * no PE transposes / attnT evacuation: pv output is already [ch, q].
#    Normalization: reciprocal of the denominator row (partition 64),
#    ones-broadcast matmul rbc[64,512] (K=1, f32r), pv evac to SBUF, DVE
#    multiply -> attnT bf16.  h0 writes attnT[0:64] directly; h1 lands in a
#    scratch tile and an SBUF->SBUF DMA shifts it to attnT[64:128] (engines
#    keep src/dst partitions aligned; DMA does the partition shift), so the
#    output projection keeps K=128 x 3 matmuls as in v2.
#  * exp paths: fused-leg Q paths are gone (with exp streaming, each leg
#    would double the pv stream).  Per-tile paths over [128,512] tiles:
#      A: ACT exp direct from PSUM
#      B: DVE evac + ACT exp from SBUF
#      F: DVE evac + DVE single-term Schraudolph
#      G: DVE evac + Pool single-term Schraudolph
#      M: DVE evac + Pool two legs + DVE add (two-term, +-1.2%)
#      N: DVE evac + Pool leg + DVE leg + DVE add
#      H: DVE evac + DVE two legs + DVE add
#  * no pair00/vproj interleave: phase A is PE-bound (69us PE vs 38us DMA),
#    so interleaving attention there buys nothing.

V3_DEFAULT_CFG = dict(
    # 32 chars indexed by granule (gi*2+h) % 32, one per [128, 2, 512] exp
    # granule (2 key tiles).  Per 8: 5 A (ACT exp) + 2 M (DVE evac + Pool
    # legs + DVE add) + 1 P (ACT evac + Pool legs + DVE add) -> per-pair
    # ACT ~24us, DVE ~19us, Pool ~21us vs PE ~28.6us.
    paths="AAMAAPMA" * 4,
    kevac="act",        # kproj evacuation
    vevac="dve",        # vproj evacuation
    oevac="dve",        # oproj evacuation
    lag=2,              # granules (2 key tiles each) between scores and pv
)


def _emit_body_v3(nc, tc, io, cfg, prof=None):
    QT, KT, VT, WQ, WK, WV, WO, BQ, OUT = (
        io["QT"], io["KT"], io["VT"], io["WQ"], io["WK"], io["WV"], io["WO"],
        io["BQ"], io["OUT"],
    )
    BF = BF16
    kev = _v2_engine(nc, cfg["kevac"])
    vev = _v2_engine(nc, cfg["vevac"])
    oev = _v2_engine(nc, cfg["oevac"])
    LAG = int(cfg["lag"])
    NKT = S // 128          # 32 key tiles

    consts = tc.alloc_tile_pool(name="consts", bufs=1)
    big = tc.alloc_tile_pool(name="big", bufs=1)

    wq_t, wk_t, wv_t = [], [], []
    for i in range(6):
        wk = consts.tile([128, C], BF, tag=f"wk{i}", name=f"wk{i}")
        d0 = nc.sync.dma_start(out=wk, in_=WK[ts(i, 128), :])
        if prof is not None and i == 0:
            prof.snap(0, d0)
        wk_t.append(wk)
    for i in range(6):
        wq = consts.tile([128, C], BF, tag=f"wq{i}", name=f"wq{i}")
        nc.scalar.dma_start(out=wq, in_=WQ[ts(i, 128), :])
        wq_t.append(wq)
    # late-consumed constants: allocate now, DMA after the first kproj chunk
    wv_t = [consts.tile([128, C], BF, tag=f"wv{i}", name=f"wv{i}")
            for i in range(6)]
    wo_t = [consts.tile([128, D_MODEL], BF, tag=f"wo{p}", name=f"wo{p}")
            for p in range(3)]
    bq_t = [consts.tile([128, 1], F32, tag=f"bq{p}", name=f"bq{p}")
            for p in range(3)]

    def _emit_late_const_dmas():
        for i in range(6):
            nc.scalar.dma_start(out=wv_t[i], in_=WV[ts(i, 128), :])
        for p in range(3):
            nc.sync.dma_start(out=wo_t[p], in_=WO[ts(p, 128), :])
            nc.scalar.dma_start(
                out=bq_t[p],
                in_=BQ[ts(p, 128)].rearrange("(p one) -> p one", one=1))

    ones6 = consts.tile([128, NHL], BF, tag="ones6", name="ones6")
    nc.vector.memset(ones6, 1.0)
    # bf16 ones row at partition 64 (aligned with the pv denominator row)
    # for the K=1 broadcast matmul (bf16 -> 1 cyc/row on the PE)
    onesr = consts.tile([128, 64], BF, tag="onesr", name="onesr")
    nc.vector.memset(onesr[ds(64, 1), :], 1.0)

    qT_t = [big.tile([128, QR], BF, tag=f"qT{p}", name=f"qT{p}") for p in range(3)]
    kT_t = [big.tile([128, S], BF, tag=f"kT{p}", name=f"kT{p}") for p in range(3)]
    vext_t = [
        big.tile([128, NHL, HD + 1], BF, tag=f"vx{j}", name=f"vx{j}")
        for j in range(NKT)
    ]

    # attention pools before projection pools (LIFO release order)
    expp = tc.alloc_tile_pool(name="expp", bufs=3)
    attnTp = tc.alloc_tile_pool(name="attnTp", bufs=4)
    smallp = tc.alloc_tile_pool(name="smallp", bufs=2)
    outp = tc.alloc_tile_pool(name="outp", bufs=2)
    # PSUM budget is exactly 8 banks.  Phase A: psSa (2x2 banks) + psPV (2)
    # + psA (2) = 8.  Phase B: psSa + psSb (allocated from the banks psA
    # frees, LIFO arena) + psPV = 8.  Scores granules cycle psSa/psSa/psSb
    # for 3-deep buffering; rbc and oproj psum also borrow these rings.
    psSa = tc.alloc_tile_pool(name="psSa", bufs=2, space="PSUM")
    psPV = tc.alloc_tile_pool(name="psPV", bufs=1, space="PSUM")

    def emit_exp(idx, nm, sp):
        """Consume a [128, 2, 512] scores psum granule -> (bf16 exp tile,
        finisher|None).  Two-term paths defer the DVE add to the finisher so
        it does not head-block the in-order DVE queue while Pool runs the
        legs; the caller invokes finishers one granule later."""
        path = cfg["paths"][idx % 32]
        e = expp.tile([128, 2, 512], BF, tag="e", name=f"e{nm}",
                      bufs=2 * LAG + 3)
        if path == "A":
            nc.scalar.activation(e, sp, AF.Exp)
            return e, None
        ssb = expp.tile([128, 2, 512], BF, tag="ssb", name=f"ssb{nm}", bufs=3)
        if path in ("P", "B"):
            nc.scalar.copy(ssb, sp)
        else:
            nc.vector.tensor_copy(ssb, sp)
        if path == "B":
            nc.scalar.activation(e, ssb, AF.Exp)
        elif path == "F":
            nc.vector.tensor_scalar(
                e.bitcast(I16), ssb, SCH_A, SCH_B0, mybir.AluOpType.mult,
                mybir.AluOpType.add)
        elif path == "G":
            nc.gpsimd.tensor_scalar(
                e.bitcast(I16), ssb, SCH_A, SCH_B0, mybir.AluOpType.mult,
                mybir.AluOpType.add)
        elif path in ("M", "N", "H", "P"):
            leg1 = nc.vector if path == "H" else nc.gpsimd
            leg2 = nc.vector if path in ("N", "H") else nc.gpsimd
            t1 = expp.tile([128, 2, 512], BF, tag="t1", name=f"t1{nm}", bufs=4)
            t2 = expp.tile([128, 2, 512], BF, tag="t2", name=f"t2{nm}", bufs=4)
            leg1.tensor_scalar(
                t1.bitcast(I16), ssb, SCH_A, SCH_B1, mybir.AluOpType.mult,
                mybir.AluOpType.add)
            leg2.tensor_scalar(
                t2.bitcast(I16), ssb, SCH_A, SCH_B2, mybir.AluOpType.mult,
                mybir.AluOpType.add)
            return e, lambda: nc.vector.tensor_add(e, t1, t2)
        else:
            raise ValueError(path)
        return e, None

    NG = NKT // 2           # 16 granules of 2 key tiles
    sp_pools = []           # filled after psSb allocation: [psSa, psSa, psSb]
    oproj_q = []            # pending oproj thunks, drained one per granule

    def make_pair_end_thunk(qc, p, h, rr, asb, attnT):
        """rbc + normalize-multiply (+ h1 partition-shift DMA); the pvh
        readers (recip/asb) were already emitted inline at the boundary."""
        def thunk():
            # rbc borrows a scores-ring psum slot (all banks accounted for)
            rbc = sp_pools[2].tile([128, 2, 512], F32, tag="psS",
                                   name=f"rbc{qc}_{p}_{h}")
            nc.tensor.matmul(
                rbc[0:64, 0, :],
                lhsT=onesr[ds(64, 1), :],
                rhs=rr[ds(64, 1), :],
                start=True, stop=True,
            )
            if h == 0:
                nc.vector.tensor_mul(attnT[0:64, :], asb[0:HD, :],
                                     rbc[0:64, 0, :])
            else:
                ath1 = smallp.tile([128, 512], BF, tag="ath1",
                                   name=f"ath1{qc}_{p}")
                nc.vector.tensor_mul(ath1[0:64, :], asb[0:HD, :],
                                     rbc[0:64, 0, :])
                gate = nc.sync.dma_start(out=attnT[ds(64, 64), :],
                                         in_=ath1[0:64, :])
                if prof is not None:
                    prof.snap(4 + qc * 3 + p, gate)
        return thunk

    class _PairState:
        def __init__(self, J, qc, p):
            self.qc, self.p = qc, p
            self.pvh = [
                psPV.tile([128, 512], F32, tag=f"pv{h}",
                          name=f"pv{qc}_{p}_{h}", bufs=1)
                for h in range(2)
            ]
            self.attnT = attnTp.tile([128, 512], BF, tag="attnT",
                                     name=f"aT{qc}_{p}")
            self.es = {}
            self.fins = {}

    def attn_phase(jobs, attnTs_out):
        """One continuous granule stream across all (qc, p) pairs: scores at
        step s, two-term adds at s+1, pv at s+LAG.  Pair tails overlap the
        next pair's head, so PE never drains at pair boundaries."""
        NSTEP = len(jobs) * NG
        state = {}

        def emit_scores(s):
            J, g = divmod(s, NG)
            qc, p = jobs[J]
            st = state.get(J)
            if st is None:
                st = state[J] = _PairState(J, qc, p)
                attnTs_out.append(st.attnT)
            for h in range(2):
                sp = sp_pools[(g * 2 + h) % 3].tile(
                    [128, 2, 512], F32, tag="psS", name=f"sp{qc}_{p}_{g}_{h}")
                for i in range(2):
                    nc.tensor.matmul(
                        sp[:, i, :],
                        lhsT=kT_t[p][ds(64 * h, 64), ts(2 * g + i, 128)],
                        rhs=qT_t[p][ds(64 * h, 64), ts(qc, 512)],
                        start=True, stop=True,
                    )
                st.es[(g, h)], fin = emit_exp(
                    g * 2 + h, f"{qc}_{p}_{g}_{h}", sp)
                if fin is not None:
                    st.fins[(g, h)] = fin

        def emit_pv(s):
            J, g = divmod(s, NG)
            st = state[J]
            qc, p = st.qc, st.p
            for i in range(2):
                j = 2 * g + i
                for h in range(2):
                    nc.tensor.matmul(
                        st.pvh[h][0:HD + 1, :],
                        lhsT=vext_t[j][:, p * 2 + h, :],
                        rhs=st.es[(g, h)][:, i, :],
                        start=(j == 0), stop=(j == NKT - 1),
                    )
            for h in range(2):
                st.es.pop((g, h))
            if g == NG - 1:
                # pair done: emit the pvh readers (recip + evac) inline so
                # the psPV slots free for the next pair; defer rbc/mult/DMA
                for h in range(2):
                    rr = smallp.tile([128, 512], BF, tag="rr",
                                     name=f"rr{qc}_{p}_{h}")
                    with nc.allow_low_precision(
                            reason="bf16 softmax-denominator reciprocal "
                                   "(0.4% on the normalization scale)"):
                        nc.vector.reciprocal(rr[ds(64, 1), :],
                                             st.pvh[h][ds(HD, 1), :])
                    asb = smallp.tile([128, 512], F32, tag="asb",
                                      name=f"asb{qc}_{p}_{h}")
                    nc.vector.tensor_copy(asb[0:HD, :], st.pvh[h][0:HD, :])
                    oproj_q.append(
                        make_pair_end_thunk(qc, p, h, rr, asb, st.attnT))
                if p == 2:
                    for qs in range(4):
                        oproj_q.append(
                            make_oproj_thunk(qc, qs,
                                             attnTs_out[qc * 3:qc * 3 + 3]))
                del state[J]

        for s in range(NSTEP + LAG):
            if oproj_q:
                oproj_q.pop(0)()
            if s < NSTEP:
                emit_scores(s)
            if s >= 1 and s - 1 < NSTEP:
                J1, g1 = divmod(s - 1, NG)
                for h in range(2):
                    f = state[J1].fins.pop((g1, h), None)
                    if f is not None:
                        f()
            if s >= LAG:
                emit_pv(s - LAG)
        while oproj_q:
            oproj_q.pop(0)()

    # ================= phase A: projections =================
    instream = tc.alloc_tile_pool(name="instream", bufs=2)
    psA = tc.alloc_tile_pool(name="psA", bufs=2, space="PSUM")

    for ck in range(S // 512):
        kin = [instream.tile([128, 512], BF, tag=f"xin{i}", name=f"kin{i}_{ck}",
                             bufs=2) for i in range(6)]
        for i in range(6):
            dq = nc.sync if i % 2 == 0 else nc.scalar
            dq.dma_start(out=kin[i], in_=KT[ts(i, 128), ts(ck, 512)])
        for p in range(3):
            ps = psA.tile([128, 512], F32, tag="psA", name=f"ps_k{p}_{ck}")
            for i in range(6):
                nc.tensor.matmul(
                    ps, lhsT=wk_t[i][:, ts(p, 128)], rhs=kin[i],
                    start=(i == 0), stop=(i == 5),
                )
            gate = _v2_copy(nc, kev, kT_t[p][:, ts(ck, 512)], ps)
            if prof is not None and ck == S // 512 - 1 and p == 2:
                prof.snap(2, gate)
        if ck == 0:
            _emit_late_const_dmas()

    for qc in range(QR // 512):
        qin = [instream.tile([128, 512], BF, tag=f"xin{i}", name=f"qin{i}_{qc}",
                             bufs=2) for i in range(6)]
        for i in range(6):
            dq = nc.sync if i % 2 == 0 else nc.scalar
            dq.dma_start(out=qin[i], in_=QT[ts(i, 128), ts(qc, 512)])
        for p in range(3):
            ps = psA.tile([128, 512], F32, tag="psA", name=f"ps_q{p}_{qc}")
            for i in range(6):
                nc.tensor.matmul(
                    ps, lhsT=wq_t[i][:, ts(p, 128)], rhs=qin[i],
                    start=(i == 0), stop=(i == 5),
                )
            gate = nc.scalar.activation(
                qT_t[p][:, ts(qc, 512)], ps, AF.Identity, bias=bq_t[p], scale=SCALE
            )
            if prof is not None and qc == QR // 512 - 1 and p == 2:
                prof.snap(1, gate)

    for ck in range(S // 512):
        vin = [instream.tile([128, 512], BF, tag=f"xin{i}", name=f"vin{i}_{ck}",
                             bufs=2) for i in range(6)]
        for i in range(6):
            dq = nc.sync if i % 2 == 0 else nc.scalar
            dq.dma_start(out=vin[i], in_=VT[ts(i, 128), ts(ck, 512)])
        for ksub in range(4):
            j = ck * 4 + ksub
            ps = psA.tile([128, 512], F32, tag="psA", name=f"ps_v{j}")
            for i in range(6):
                nc.tensor.matmul(
                    ps[:, 0:C], lhsT=vin[i][:, ts(ksub, 128)], rhs=wv_t[i],
                    start=(i == 0), stop=(i == 5),
                )
            _v2_copy(nc, vev,
                vext_t[j][:, :, 0:HD], ps[:, 0:C].rearrange("p (h d) -> p h d", h=NHL))
            gate = _v2_copy(nc, vev, vext_t[j][:, :, HD], ones6)
            if prof is not None and j == NKT - 1:
                prof.snap(3, gate)

    psA.release()
    instream.release()

    # psSb reuses the 2 PSUM banks psA freed (LIFO arena); scores granules
    # cycle psSa/psSa/psSb for 3-deep buffering.
    psSb = tc.alloc_tile_pool(name="psSb", bufs=1, space="PSUM")
    sp_pools.extend([psSa, psSa, psSb])

    def make_oproj_thunk(qc, qs, attnTs):
        def thunk():
            ob = outp.tile([128, D_MODEL], F32, tag="ob", name=f"ob{qc}_{qs}")
            po = psSa.tile([128, 2, 512], F32, tag="psS", name=f"po{qc}_{qs}")
            for oc in range(2):
                for p in range(3):
                    nc.tensor.matmul(
                        po[:, oc, 0:384],
                        lhsT=attnTs[p][:, ts(qs, 128)],
                        rhs=wo_t[p][:, ts(oc, 384)],
                        start=(p == 0), stop=(p == 2),
                    )
                _v2_copy(nc, oev, ob[:, ts(oc, 384)], po[:, oc, 0:384])
            gate = nc.sync.dma_start(out=OUT[ds(qc * 512 + qs * 128, 128), :], in_=ob)
            if prof is not None and qs == 3:
                prof.snap(10 + qc, gate)
        return thunk

    # ================= phase B: attention =================
    jobs = [(qc, p) for qc in range(QR // 512) for p in range(3)]
    attnTs_all = []
    attn_phase(jobs, attnTs_all)

    for pool in [psSb, psPV, psSa, outp, smallp, attnTp, expp, big, consts]:
        pool.release()


_nc_cache = {}


PROF_LK = 256           # ladder length (ticks)
PROF_TICK_CYC = 4800    # NX cycles per tick  (~4us at 1.2 GHz)
PROF_NSNAP = 12


class _Prof:
    """On-device sampling profiler: a GPSIMD tick ladder (sequencer-only
    stores + fixed-cycle nops, invisible to Tile's dep tracker) plus snapshot
    DMAs of the tick buffer gated on phase-completion instructions."""

    def __init__(self, nc, prog_ap, PROG):
        self.nc = nc
        self.prog_ap = prog_ap
        self.PROG = PROG

    def snap(self, idx, gate):
        from concourse.tile_rust import add_dep_helper
        d = self.nc.sync.dma_start(out=self.PROG[ds(idx, 1), :], in_=self.prog_ap)
        add_dep_helper(d.ins, gate.ins, sync=True, reason=f"prof snap {idx}")


def _emit_prof_ladder(nc, prog_ap):
    """Emit (post-Tile) the Pool tick ladder, then relocate it to just after
    Pool's preamble-barrier instructions so it runs concurrently with the
    kernel body."""
    ladder = []
    reg_ctx = nc.gpsimd.register("prof_tick")
    reg = reg_ctx.__enter__()
    z = nc.gpsimd.reg_alu(reg, 0, 0, mybir.AluOpType.add)
    ladder.append(z.ins)
    for i in range(PROF_LK):
        s = nc.gpsimd.store(prog_ap[0:1, ds(i, 1)], reg)
        ladder.append(s.ins)
    for i in range(PROF_LK):
        a = nc.gpsimd.reg_alu(reg, reg, 1, mybir.AluOpType.add)
        ladder.append(a.ins)
        s = nc.gpsimd.store(prog_ap[0:1, ds(i, 1)], reg)
        ladder.append(s.ins)
        n = nc.gpsimd.nop(cycle_cnt=PROF_TICK_CYC, nofuse=True)
        ladder.append(n.ins)
    ladder_set = set(id(x) for x in ladder)
    f = nc.m.functions[0]
    # remove from wherever they were appended
    for blk in f.blocks:
        blk.instructions = [x for x in blk.instructions if id(x) not in ladder_set]
    # insert at the start of the TileContext body block so Pool runs the
    # ladder concurrently with the kernel (Pool is otherwise unused there)
    for blk in f.blocks:
        if blk.name.startswith("tile_context"):
            blk.instructions[0:0] = ladder
            return
    raise RuntimeError("profiler: no tile_context block found for tick ladder")


def build_nc(reps=1, use_f32r=True, split_waits=True, stages=("proj", "attn", "oproj"),
             timing_mode=False, att_bf16=False, in_bf16=False, profile_ladder=False,
             body="v3", cfg_over=None):
    cfg = dict(V3_DEFAULT_CFG) if body == "v3" else dict(V2_DEFAULT_CFG)
    if profile_ladder and body == "v2":
        # the tick ladder owns the Pool engine: route all Pool work to DVE.
        # The measured time is therefore an upper bound of the production
        # kernel (which spreads the same work over one more engine).
        cfg["paths"] = (cfg["paths"].replace("G", "F").replace("C", "H")
                        .replace("M", "H").replace("N", "H"))
        cfg["q_legs_dve"] = True
    if profile_ladder and body == "v3":
        # tick ladder owns Pool: two-term legs go all-DVE, P -> exact ACT exp
        cfg["paths"] = (cfg["paths"].replace("G", "F").replace("M", "H")
                        .replace("N", "H").replace("P", "B"))
    if cfg_over:
        cfg.update(cfg_over)
    key = (reps, use_f32r, split_waits, tuple(stages), timing_mode, att_bf16, in_bf16,
           profile_ladder, body, tuple(sorted(cfg.items())))
    if key in _nc_cache:
        return _nc_cache[key]
    nc = bass.Bass()
    if body in ("v2", "v3"):
        DI = mybir.dt.float8e4 if cfg.get("fp8_proj") else BF16
        DA = BF16
        DW = BF16
    else:
        DT = F32R if use_f32r else F32
        DA = BF16 if att_bf16 else DT
        DI = BF16 if in_bf16 else DT
        DW = DT
    io = {
        "QT": nc.declare_dram_parameter("QT", [D_MODEL, QR], DI, isOutput=False),
        "KT": nc.declare_dram_parameter("KT", [D_MODEL, S], DI, isOutput=False),
        "VT": nc.declare_dram_parameter("VT", [D_MODEL, S], DI, isOutput=False),
        "WQ": nc.declare_dram_parameter("WQ", [D_MODEL, C], DI, isOutput=False),
        "WK": nc.declare_dram_parameter("WK", [D_MODEL, C], DI, isOutput=False),
        "WV": nc.declare_dram_parameter("WV", [D_MODEL, C], DI, isOutput=False),
        "WO": nc.declare_dram_parameter("WO", [C, D_MODEL], DW, isOutput=False),
        "BQ": nc.declare_dram_parameter("BQ", [C], F32, isOutput=False),
        "OUT": nc.declare_dram_parameter("OUT", [QR, D_MODEL], F32, isOutput=True),
    }
    if body == "v2":
        io["EYE"] = nc.declare_dram_parameter("EYE", [128, 128], F32R, isOutput=False)
        if cfg.get("debug"):
            io["DBG"] = nc.declare_dram_parameter("DBG", [128, 1024], F32,
                                                  isOutput=True)
    prof = None
    prog_ap = None
    if profile_ladder:
        PROG = nc.declare_dram_parameter(
            "PROG", [PROF_NSNAP, PROF_LK], mybir.dt.int32, isOutput=True)
        prog_ap = nc.alloc_sbuf_tensor("prog_ticks", [1, PROF_LK], mybir.dt.int32).ap()
        prof = _Prof(nc, prog_ap, PROG)
    with TileContext(nc) as tc:
        for _ in range(reps):
            if body == "v3":
                _emit_body_v3(nc, tc, io, cfg, prof=prof)
            elif body == "v2":
                _emit_body_v2(nc, tc, io, cfg, prof=prof)
            else:
                _emit_body(nc, tc, io, use_f32r=use_f32r, stages=stages,
                           att_bf16=att_bf16, in_bf16=in_bf16, prof=prof)
    if profile_ladder:
        _emit_prof_ladder(nc, prog_ap)
    if split_waits:
        _split_excess_waits(nc)
    _nc_cache[key] = nc
    return nc


def fp8_proj_enabled():
    return bool(V2_DEFAULT_CFG.get("fp8_proj"))


def make_in_maps(Q, K, V, Wq, bq, Wk, bk, Wv, bv, Wo, bo, in_bf16=False,
                 body="v3"):
    """Host-side sharding.  Returns (in_maps, host_const) where host_const is
    the [768] vector added to every output row (bv @ Wo + bo)."""
    Qm = np.asarray(Q, np.float32).reshape(S, D_MODEL)
    Km = np.asarray(K, np.float32).reshape(S, D_MODEL)
    Vm = np.asarray(V, np.float32).reshape(S, D_MODEL)
    QT = np.ascontiguousarray(Qm.T)
    KT = np.ascontiguousarray(Km.T)
    VT = np.ascontiguousarray(Vm.T)
    Wq = np.asarray(Wq, np.float32); Wk = np.asarray(Wk, np.float32)
    Wv = np.asarray(Wv, np.float32); Wo = np.asarray(Wo, np.float32)
    bq = np.asarray(bq, np.float32); bv = np.asarray(bv, np.float32)
    bo = np.asarray(bo, np.float32)

    import ml_dtypes
    if body in ("v2", "v3"):
        in_bf16 = True
    fp8_proj = body == "v2" and fp8_proj_enabled()
    def cvt(a):
        if fp8_proj:
            return np.ascontiguousarray(a).astype(ml_dtypes.float8_e4m3)
        return np.ascontiguousarray(a).astype(ml_dtypes.bfloat16) if in_bf16 \
            else np.ascontiguousarray(a)
    def cvt_w(a):
        if body in ("v2", "v3"):
            return np.ascontiguousarray(a).astype(ml_dtypes.bfloat16)
        return np.ascontiguousarray(a)
    in_maps = []
    for c in range(NCORES):
        g, b = divmod(c, QB)
        ch = slice(g * C, (g + 1) * C)
        m = {
            "QT": cvt(QT[:, b * QR : (b + 1) * QR]),
            "KT": cvt(KT),
            "VT": cvt(VT),
            "WQ": cvt(Wq[:, ch]),
            "WK": cvt(Wk[:, ch]),
            "WV": cvt(Wv[:, ch]),
            "WO": cvt_w(Wo[ch, :]),
            # device computes qT = psum*SCALE + BQ, so prescale the bias here
            "BQ": np.ascontiguousarray(bq[ch] * np.float32(SCALE)),
        }
        if body == "v2":
            m["EYE"] = np.eye(128, dtype=np.float32)
        in_maps.append(m)
    host_const = (bv @ Wo + bo).astype(np.float32)
    return in_maps, host_const


def kernel(Q, K, V, Wq, bq, Wk, bk, Wv, bv, Wo, bo):
    nc = build_nc()
    in_maps, host_const = make_in_maps(Q, K, V, Wq, bq, Wk, bk, Wv, bv, Wo, bo)
    res = run_bass_kernel_spmd(nc, in_maps, core_ids=list(range(NCORES)))
    out = np.zeros((S, D_MODEL), np.float32)
    for c in range(NCORES):
        g, b = divmod(c, QB)
        out[b * QR : (b + 1) * QR, :] += res.results[c]["OUT"]
    out += host_const[None, :]
    return out.reshape(1, S, D_MODEL)

